# revision 75
# baseline (speedup 1.0000x reference)
"""Trainium2 Bass kernel for a pre-norm transformer block (B=8, N=1024, C=768).

Data-parallel over batch: each of the 8 NeuronCores runs the full block for
one batch element. Activations are feature-major ([feat, tok]) so matmuls
contract over the partition dim with no on-device transposes.

vs the f32r baseline:
  - bf16 matmul operands end-to-end (PSUM accumulation stays f32); rel err
    ~1e-3, well under the 2e-2 gate. Halves DMA and enables DVE 2x modes.
  - LayerNorm gain/bias folded into the downstream weights/biases on the
    host, so normalize is 2 DVE ops (sub, mul) per tile; squares for the
    sumsq stat run on the ACT engine. rstd = Exp(-0.5*Ln(var+eps)) keeps the
    whole program inside the natural_log_exp ACT table (only the fc1 Gelu
    switches tables), so no table load lands on the LN critical path.
  - q/k biases ride the PSUM->SBUF move as ACT Identity ops with a
    per-partition bias AP; proj/fc2 biases fuse into the residual add as
    scalar_tensor_tensor on DVE. Only the v/LN stat ones-matmuls remain.
  - Wide [128,1024] 2-bank PSUM tiles: one exp per head-PAIR per key-chunk,
    one gelu per fc1 row-pair, halving ACT instruction count.
  - Attention is software-pipelined: scores(kc) emit before PV(kc-1). Pairs
    run in order [5,0,1,2,3,4] and proj contracts chunks in the same order,
    so the last pair's normalize overlaps the first 5/6 of proj's matmuls.
  - fc1/fc2 weights stream once per rep (full-width token processing), not
    once per token-half; fc2 emits half-width outputs so the tail drain is
    one [128,512] DVE+DMA, not [128,1024].
"""

import os
import sys

import numpy as np

for _p in ("/opt/trn_rl_repo", "/root/.axon_site/_ro/trn_rl_repo"):
    if os.path.isdir(_p) and _p not in sys.path:
        sys.path.append(_p)

import concourse.bass as bass  # noqa: E402
import concourse.tile as tile  # noqa: E402
from concourse import bacc, mybir  # noqa: E402
from concourse.bass_utils import run_bass_kernel_spmd  # noqa: E402

F32 = mybir.dt.float32
BF16 = mybir.dt.bfloat16
AF = mybir.ActivationFunctionType
OP = mybir.AluOpType

P = 128
D = 768
KD = D // P          # 6 feature chunks over the 768 contraction dim
NTOK = 1024
F = 512              # token-half width (matmul free dim)
NHALF = NTOK // F    # 2
H = 12
DK = 64
DFF = 3072
MF1 = DFF // P       # 24
TC = NTOK // P       # 8 key/token chunks
EPS = 1e-5
N_CORES = 8

P_ORDER = (5, 0, 1, 2, 3, 4)  # attention pair order; proj contracts likewise


def build_program(reps=1):
    stage = int(os.environ.get("KERNEL_STAGE", "5"))
    nc = bacc.Bacc(
        "TRN2", target_bir_lowering=False, debug=False, num_devices=N_CORES
    )

    din = lambda name, shape, dt=BF16: nc.dram_tensor(
        name, shape, dt, kind="ExternalInput"
    ).ap()
    xt = din("xt", [P, KD, NTOK])
    onesb = din("onesb", [P, 1])
    wqk = din("wqk", [2 * KD, P, KD, P])
    wv = din("wv", [P, KD, KD, P])
    bqk_pp = din("bqk_pp", [P, 2 * KD], F32)     # q/k bias, [part, chunk]
    wqk_rsum = din("wqk_rsum", [1, 2 * D])       # NEGATED q/k row-sums
    wv_rsum = din("wv_rsum", [1, D])             # NEGATED v row-sums
    wproj = din("wproj", [P, KD, KD, P])
    bproj_pp = din("bproj_pp", [P, KD], F32)
    wfc1 = din("wfc1", [MF1, P, KD, P])
    bfc1 = din("bfc1", [P, MF1], F32)
    wfc2 = din("wfc2", [KD, P, MF1, P])
    bfc2_pp = din("bfc2_pp", [P, KD], F32)
    yt = nc.dram_tensor("yt", [P, KD, NTOK], F32, kind="ExternalOutput").ap()
    # DRAM bounce row for the [1,F] -> [tok_part, chunk] rstd transpose
    # (SBUF->SBUF partition-crossing APs don't balance; DRAM APs are free).
    rs_scr = nc.dram_tensor("rs_scr", [1, NTOK], F32, kind="Internal").ap()

    with tile.TileContext(nc) as tc:
        # PSUM pools are all per-phase (8-bank budget): qkv spp(3 wide),
        # attention spA(2 wide)+opsw(2 wide), MLP mlpw(4 wide).
        # ---- constant/global SBUF pools ----
        const = tc.alloc_tile_pool(name="const", bufs=1)
        stat = tc.alloc_tile_pool(name="stat", bufs=5)
        bc1 = tc.alloc_tile_pool(name="bc1", bufs=4)     # [1,F] bf16 casts
        bcP = tc.alloc_tile_pool(name="bcP", bufs=4)     # [P,F] bf16 bcasts
        bcR = tc.alloc_tile_pool(name="bcR", bufs=2)     # [DK,2F] f32 bcasts
        sqp = tc.alloc_tile_pool(name="sqp", bufs=12)
        tmp = tc.alloc_tile_pool(name="tmp", bufs=3)
        ptp = tc.alloc_tile_pool(name="ptp", bufs=4)
        outp = tc.alloc_tile_pool(name="outp", bufs=3)
        wstream = tc.alloc_tile_pool(name="wstream", bufs=4)
        f1s = tc.alloc_tile_pool(name="f1s", bufs=8)
        f2s = tc.alloc_tile_pool(name="f2s", bufs=2)

        eps_sb = const.tile([1, 1], F32)
        nc.vector.memset(eps_sb, EPS)
        onesrow = const.tile([1, NTOK], BF16)
        nc.vector.memset(onesrow, 1.0)
        onesb_sb = const.tile([P, 1], BF16, name="onesb_sb")
        nc.sync.dma_start(out=onesb_sb[:], in_=onesb[:])

        def load_const(ap_dram, shape=None, dt=None):
            t = const.tile(
                shape or list(ap_dram.shape), dt or ap_dram.dtype,
                name=ap_dram.name + "_sb",
            )
            nc.sync.dma_start(out=t[:], in_=ap_dram[:])
            return t

        # small biases up front; the two big weight consts (wv, wproj) are
        # deferred until after rep-0's xt DMA so LN1 isn't starved.
        bqk_sb = load_const(bqk_pp)
        wqk_rsum_sb = load_const(wqk_rsum)
        wv_rsum_sb = load_const(wv_rsum)
        bproj_sb = load_const(bproj_pp)
        bfc1_sb = load_const(bfc1)
        bfc2_sb = load_const(bfc2_pp)
        deferred = {}

        # ---- layernorm: stats via ones-matmuls (one wide PSUM tile:
        # sum in bank 0, sumsq in bank 1), 2-op normalize.
        # rstd = Exp(-0.5*Ln(var+eps)) -- stays in the nat_log_exp table ----
        def emit_ln_stats(src, half, pool, sqs=None, tag="wd"):
            cols = slice(half * F, (half + 1) * F)
            st_ps = pool.tile([P, 2 * F], F32, tag=tag, name="st_ps")
            # Squares on DVE (bf16 2x mode, ~0.27us each) so the sumsq pass
            # never waits on the ACT queue.
            if sqs is None:
                sqs = []
                for kk in range(KD):
                    sq = sqp.tile([P, F], BF16, tag="sq", name="sq")
                    nc.vector.tensor_mul(
                        sq[:], src[:, kk, cols], src[:, kk, cols]
                    )
                    sqs.append(sq)
            for kk in range(KD):
                nc.tensor.matmul(
                    st_ps[0:1, 0:F], onesb_sb[:], src[:, kk, cols],
                    start=(kk == 0), stop=(kk == KD - 1),
                )
            for kk in range(KD):
                nc.tensor.matmul(
                    st_ps[0:1, F : 2 * F], onesb_sb[:], sqs[kk][:],
                    start=(kk == 0), stop=(kk == KD - 1),
                )
            mu16 = bc1.tile([1, F], BF16, tag="b1", name="mu16")
            nc.vector.tensor_scalar_mul(mu16[:], st_ps[0:1, 0:F], 1.0 / D)
            e2 = stat.tile([1, F], F32, tag="st", name="e2")
            nc.vector.tensor_scalar_mul(e2[:], st_ps[0:1, F : 2 * F], 1.0 / D)
            m2 = stat.tile([1, F], F32, tag="st", name="m2")
            nc.vector.tensor_mul(m2[:], mu16[:], mu16[:])
            nc.vector.tensor_tensor(e2[:], e2[:], m2[:], OP.subtract)
            sd = stat.tile([1, F], F32, tag="st", name="sd")
            nc.scalar.activation(sd[:], e2[:], AF.Sqrt, bias=eps_sb[0:1])
            rs = stat.tile([1, F], F32, tag="st", name="rs")
            nc.vector.reciprocal(rs[:], sd[:])
            rs16 = bc1.tile([1, F], BF16, tag="b1", name="rs16")
            nc.vector.tensor_copy(out=rs16[:], in_=rs[:])
            mu_b = bcP.tile([P, F], BF16, tag="bP", name="mu_b")
            nc.gpsimd.partition_broadcast(mu_b[:], mu16[:])
            rs_b = bcP.tile([P, F], BF16, tag="bP", name="rs_b")
            nc.gpsimd.partition_broadcast(rs_b[:], rs16[:])
            return mu16, rs16, mu_b, rs_b, rs

        def emit_ln_norm(src, dst, half, mu_b, rs_b):
            cols = slice(half * F, (half + 1) * F)
            for kk in range(KD):
                t1 = tmp.tile([P, F], BF16, tag="t", name="t1")
                nc.vector.tensor_tensor(
                    t1[:], src[:, kk, cols], mu_b[:], OP.subtract
                )
                nc.vector.tensor_mul(dst[:, kk, cols], t1[:], rs_b[:])

        def emit_ln(src, dst, pool, sqs0=None):
            # stats for both halves first (h1's squares never queue behind
            # h0's normalize on DVE; h1's Sqrt lands before downstream ACT
            # ops so the table never thrashes), then the normalizes.
            s0 = emit_ln_stats(src, 0, pool, sqs0)
            s1 = emit_ln_stats(src, 1, pool)
            emit_ln_norm(src, dst, 0, s0[2], s0[3])
            emit_ln_norm(src, dst, 1, s1[2], s1[3])

        for _rep in range(reps):
            spp = tc.alloc_tile_pool(name="spp", bufs=3, space="PSUM")

            xt_pool = tc.alloc_tile_pool(name="xt", bufs=1)
            attn_pool = tc.alloc_tile_pool(name="attn", bufs=1)
            qk_pool = tc.alloc_tile_pool(name="qk", bufs=12)
            vaug_pool = tc.alloc_tile_pool(name="vaug", bufs=1)
            h_pool = tc.alloc_tile_pool(name="h", bufs=1)

            xt_sb = xt_pool.tile([P, KD, NTOK], BF16, name="xt_sb")
            hT = h_pool.tile([P, KD, NTOK], BF16, name="hT")
            attnT = attn_pool.tile([P, KD, NTOK], BF16, name="attnT")
            v_aug = vaug_pool.tile([P, TC, H, DK + 1], BF16, name="v_aug")

            qk_tiles = {}

            # ---- q/k chunk: 12 matmuls -> ACT Identity+bias move to bf16 ----
            def qk_chunk_ops(m):
                wt = wstream.tile([P, KD, P], BF16, tag="w", name="wt")
                nc.sync.dma_start(out=wt[:], in_=wqk[m])
                qkt = qk_pool.tile([P, NTOK], BF16, tag="qkt", name="qkt")
                qk_tiles[m] = qkt
                holder = {}

                def group(half):
                    if half == 0:
                        holder["ps"] = spp.tile(
                            [P, 2 * F], F32, tag="wd", name="qkps"
                        )
                    c0 = half * F
                    for kk in range(KD):
                        nc.tensor.matmul(
                            holder["ps"][:, c0 : c0 + F],
                            wt[:, kk, :], hT[:, kk, c0 : c0 + F],
                            start=(kk == 0), stop=(kk == KD - 1 and half == 1),
                        )

                def move():
                    nc.scalar.activation(
                        qkt[:, :], holder["ps"][:, :], AF.Identity,
                        bias=bqk_sb[:, m : m + 1],
                    )

                return group, move

            # ---- k chunk via LN bypass: W(x-mu)*rs computed as
            # (W@x - wsum (x) mu) * rs, consuming RAW x. The key-side bias
            # cancels in softmax, so this is exact -- and it frees the k/v
            # stream from the LN1 normalize chain entirely. ----
            def emit_k_bypass(m, mu16s, rs_bs):
                wt = wstream.tile([P, KD, P], BF16, tag="w", name="wt")
                nc.sync.dma_start(out=wt[:], in_=wqk[m])
                kt = qk_pool.tile([P, NTOK], BF16, tag="qkt", name="qkt")
                qk_tiles[m] = kt
                ps = spp.tile([P, 2 * F], F32, tag="wd", name="kps")
                for half in range(NHALF):
                    c0 = half * F
                    for kk in range(KD):
                        nc.tensor.matmul(
                            ps[:, c0 : c0 + F],
                            wt[:, kk, :], xt_sb[:, kk, c0 : c0 + F],
                            start=(kk == 0), stop=False,
                        )
                    nc.tensor.matmul(
                        ps[:, c0 : c0 + F],
                        wqk_rsum_sb[0:1, m * P : (m + 1) * P],
                        mu16s[half][:],
                        start=False, stop=True,
                    )
                for half in range(NHALF):
                    c0 = half * F
                    nc.vector.tensor_mul(
                        kt[:, c0 : c0 + F], ps[:, c0 : c0 + F], rs_bs[half][:]
                    )

            # ---- v chunk (token-major), same bypass; the per-token rstd
            # rides the ACT Copy's scale port (v bias is folded into the
            # proj bias on the host: softmax weights sum to 1) ----
            def emit_v(t, mu16s, rs_tok):
                half = t * P // F
                trange = slice(t * P, (t + 1) * P)
                lt = slice(t * P - half * F, (t + 1) * P - half * F)
                ps = spp.tile([P, 2 * F], F32, tag="wd", name="vps")
                for c0, w in ((0, 512), (512, 256)):
                    for kk in range(KD):
                        nc.tensor.matmul(
                            ps[:, c0 : c0 + w],
                            xt_sb[:, kk, trange],
                            wv_sb[:, kk, c0 // P : (c0 + w) // P, :],
                            start=(kk == 0), stop=False,
                        )
                    nc.tensor.matmul(
                        ps[:, c0 : c0 + w],
                        mu16s[half][0:1, lt],
                        wv_rsum_sb[0:1, c0 : c0 + w],
                        start=False, stop=True,
                    )
                nc.scalar.activation(
                    v_aug[:, t, :, 0:DK],
                    ps[:, 0:D].rearrange("p (h d) -> p h d", d=DK),
                    AF.Copy,
                    scale=rs_tok[:, t : t + 1],
                )

            # DMA issue order feeds compute in need-order: xt h0 (LN1-h0),
            # pair-5 k weights, xt h1, then wv; wproj is deferred to the
            # attention phase.
            for kk in range(KD):
                nc.sync.dma_start(out=xt_sb[:, kk, 0:F], in_=xt[:, kk, 0:F])
            s0 = emit_ln_stats(xt_sb, 0, spp)
            for kk in range(KD):
                nc.sync.dma_start(
                    out=xt_sb[:, kk, F : 2 * F], in_=xt[:, kk, F : 2 * F]
                )
            if _rep == 0:
                deferred["wv_sb"] = load_const(wv)
            wv_sb = deferred["wv_sb"]
            nc.vector.memset(v_aug[:, :, :, DK : DK + 1], 1.0)

            s1 = emit_ln_stats(xt_sb, 1, spp)
            mu16s = (s0[0], s1[0])
            rs_bs = (s0[3], s1[3])
            # transpose rstd to token-major [tok_part, chunk] for the v moves
            rs_tok = bc1.tile([P, TC], F32, tag="rt", name="rs_tok")
            for half, s in ((0, s0), (1, s1)):
                cols = slice(half * F, (half + 1) * F)
                nc.sync.dma_start(out=rs_scr[0:1, cols], in_=s[4][:])
                nc.sync.dma_start(
                    out=rs_tok[:, half * 4 : (half + 1) * 4],
                    in_=rs_scr[0:1, cols].rearrange("a (c p) -> (a p) c", p=P),
                )

            # k and v stream off raw x while the normalize (needed only by
            # the q chunks and the interleaved pairs) completes on DVE.
            emit_k_bypass(11, mu16s, rs_bs)
            emit_k_bypass(6, mu16s, rs_bs)
            emit_v(0, mu16s, rs_tok)
            emit_v(1, mu16s, rs_tok)
            emit_ln_norm(xt_sb, hT, 0, s0[2], s0[3])
            emit_ln_norm(xt_sb, hT, 1, s1[2], s1[3])
            for t in range(2, TC):
                emit_v(t, mu16s, rs_tok)

            g_q5, mv_q5 = qk_chunk_ops(5)
            g_q5(0); g_q5(1); mv_q5()
            g_q0, mv_q0 = qk_chunk_ops(0)
            g_q0(0); g_q0(1); mv_q0()

            # Prewarm the exp table behind the last Copy move; the load
            # overlaps the first score matmuls instead of stalling the
            # mid-qkv move pipeline.
            warm1 = stat.tile([1, 1], F32, tag="st", name="warm1")
            nc.scalar.activation(warm1[:], eps_sb[:], AF.Exp)

            # Prefetch the remaining q/k chunk weights through the f1s pool
            # (idle until fc1; same tile shape, and the ring antideps line
            # up: fc1's first tiles land on slots whose attention readers
            # finish mid-attention).
            INTER = {5: (1, 7), 0: (2, 8), 1: (3, 9), 2: (4, 10)}
            wts = {}
            for w in P_ORDER[:4]:
                for m in INTER[w]:
                    wt = f1s.tile([P, KD, P], BF16, tag="f1", name="wqk_pre")
                    nc.sync.dma_start(out=wt[:], in_=wqk[m])
                    wts[m] = wt

            if _rep == 0:
                deferred["wproj_sb"] = load_const(wproj)
            wproj_sb = deferred["wproj_sb"]

            # ---- attention: double-buffered score tiles AND PV accumulators
            # so PE never couples to the ACT exp backlog ----
            spp.release()
            spA = tc.alloc_tile_pool(name="spA", bufs=2, space="PSUM")
            opsw = tc.alloc_tile_pool(name="opsw", bufs=2, space="PSUM")
            pranges = (slice(0, DK), slice(DK, P))

            # One global software pipeline over all (pair, half, kc) units:
            # S+exp for unit i, then PV for unit i-1 — no per-half drain.
            state = {}  # (j, half) -> dict(o_ps=..., pts={kc: pt})

            def emit_s_exp(j, half, kc):
                st = state.setdefault((j, half), {"pts": {}})
                if kc == 0:
                    st["o_ps"] = opsw.tile([P, 2 * F], F32, tag="ow",
                                           name="o_ps")
                q_t = qk_tiles[j]
                k_t = qk_tiles[KD + j]
                cols = slice(half * F, (half + 1) * F)
                sp = spA.tile([P, 2 * F], F32, tag="sp", name="sp")
                for hi in (0, 1):
                    pr = pranges[hi]
                    nc.tensor.matmul(
                        sp[:, hi * F : (hi + 1) * F],
                        k_t[pr, kc * P : (kc + 1) * P],
                        q_t[pr, cols],
                        start=True, stop=True,
                    )
                pt = ptp.tile([P, 2 * F], BF16, tag="pt", name="pt")
                nc.scalar.activation(
                    pt[:], sp[:], AF.Exp, scale=float(DK) ** -0.5
                )
                st["pts"][kc] = pt

            def emit_pv(j, half, kc):
                st = state[(j, half)]
                o_ps = st["o_ps"]
                pt = st["pts"].pop(kc)
                for hi in (0, 1):
                    nc.tensor.matmul(
                        o_ps[0 : DK + 1, hi * F : (hi + 1) * F],
                        v_aug[:, kc, 2 * j + hi, :],
                        pt[:, hi * F : (hi + 1) * F],
                        start=(kc == 0), stop=(kc == TC - 1),
                    )
                if kc == TC - 1:
                    # per-hi pipelined normalize: both recips (DVE), then
                    # both bcasts (Pool), then both muls (DVE) -- the in-order
                    # engine queues overlap across the two heads, halving the
                    # serial tail before the PSUM pool boundary.
                    cols = slice(half * F, (half + 1) * F)
                    recs, rbs = [], []
                    for hi in (0, 1):
                        rec = stat.tile([1, F], F32, tag="st", name="rec")
                        nc.vector.reciprocal(
                            rec[:], o_ps[DK : DK + 1, hi * F : (hi + 1) * F]
                        )
                        recs.append(rec)
                    for hi in (0, 1):
                        rec_b = bcR.tile([DK, F], F32, tag="bR", name="rec_b")
                        nc.gpsimd.partition_broadcast(rec_b[:], recs[hi][:])
                        rbs.append(rec_b)
                    for hi in (0, 1):
                        nc.vector.tensor_mul(
                            attnT[pranges[hi], j, cols],
                            o_ps[0:DK, hi * F : (hi + 1) * F],
                            rbs[hi][:],
                        )

            # Interleaved q/k chunk: 12 matmuls into a spA-ring tile fill the
            # PE while ACT works down the exp backlog; the PSUM->SBUF move
            # runs on DVE (idle during attention) so ACT never sees it.
            def emit_qk_inter(m):
                qkt = qk_pool.tile([P, NTOK], BF16, tag="qkt", name="qkt")
                qk_tiles[m] = qkt
                ps = spA.tile([P, 2 * F], F32, tag="sp", name="qkps")
                for half in range(NHALF):
                    c0 = half * F
                    for kk in range(KD):
                        nc.tensor.matmul(
                            ps[:, c0 : c0 + F],
                            wts[m][:, kk, :], hT[:, kk, c0 : c0 + F],
                            start=(kk == 0), stop=(kk == KD - 1 and half == 1),
                        )
                nc.vector.tensor_scalar_add(
                    qkt[:, :], ps[:, :], bqk_sb[:, m : m + 1]
                )

            units = [(j, half, kc)
                     for j in P_ORDER for half in range(NHALF)
                     for kc in range(TC)]
            # Emit the remaining chunks as a block here (not interleaved into
            # the unit stream: insertions starve ACT of exp backlog). The spA
            # ring + DVE moves keep them off the attention ACT queue.
            sched = {}
            for w in P_ORDER[:4]:
                for m in INTER[w]:
                    emit_qk_inter(m)

            # fc1/fc2 first-weight DMAs: emitted now (all readers of the f1s
            # slots they rotate into are recorded), so the transfers run
            # during attention and the MLP start is never DMA-paced.
            f1_tiles = []
            for m in range(KD):
                wt = f1s.tile([P, KD, P], BF16, tag="f1", name="f1w")
                nc.sync.dma_start(out=wt[:], in_=wfc1[m])
                f1_tiles.append(wt)
            w2_first = f2s.tile([P, MF1, P], BF16, tag="f2", name="f2w")
            nc.sync.dma_start(out=w2_first[:], in_=wfc2[0])
            SKEW = 1  # PE stays 1 S+exp unit ahead of the PV stream
            for i, u in enumerate(units):
                emit_s_exp(*u)
                if i >= SKEW:
                    emit_pv(*units[i - SKEW])
                if i in sched:
                    emit_qk_inter(sched[i])
            for u in units[-SKEW:]:
                emit_pv(*u)

            # Prewarm the sqrt table for LN2 behind the last exp; the load
            # overlaps proj's matmuls instead of the LN2 chain.
            warm2 = stat.tile([1, 1], F32, tag="st", name="warm2")
            nc.scalar.activation(warm2[:], eps_sb[:], AF.Sqrt)

            h_pool.release()
            vaug_pool.release()
            qk_pool.release()

            # ---- proj + residual -> x2T; contraction follows P_ORDER so the
            # last pair's normalize overlaps the first 5 chunks' matmuls.
            # m0..m3 borrow the attention pools' PSUM tiles (their per-buffer
            # antideps resolve pair-by-pair), so the pool release boundary
            # never parks the PE behind the last pair's normalize chain ----
            x2_pool = tc.alloc_tile_pool(name="x2", bufs=1, side="right")
            h2_pool = tc.alloc_tile_pool(name="h2", bufs=1, side="right")
            g_pool = tc.alloc_tile_pool(name="g", bufs=1, side="right")
            x2T = x2_pool.tile([P, KD, NTOK], BF16, name="x2T")
            h2T = h2_pool.tile([P, KD, NTOK], BF16, name="h2T")
            gT = g_pool.tile([P, MF1, NTOK], BF16, name="gT")

            def emit_proj_m(m, ps):
                for half in range(NHALF):
                    c0 = half * F
                    for i, kk in enumerate(P_ORDER):
                        nc.tensor.matmul(
                            ps[:, c0 : c0 + F],
                            wproj_sb[:, kk, m, :],
                            attnT[:, kk, c0 : c0 + F],
                            start=(i == 0), stop=(i == KD - 1),
                        )
                # LN2 squares for this chunk ride right behind the stt on
                # DVE, so LN2 stats never wait on a square backlog. The last
                # chunk's stt is split per half so its h0 lands sooner (it
                # gates the LN2-h0 sum pass).
                if m == KD - 1:
                    for half in range(NHALF):
                        c0 = half * F
                        nc.vector.scalar_tensor_tensor(
                            x2T[:, m, c0 : c0 + F], ps[:, c0 : c0 + F],
                            bproj_sb[:, m : m + 1],
                            xt_sb[:, m, c0 : c0 + F], OP.add, OP.add,
                        )
                        sq = sqp.tile([P, F], BF16, tag="sq", name="sq")
                        nc.vector.tensor_mul(
                            sq[:], x2T[:, m, c0 : c0 + F],
                            x2T[:, m, c0 : c0 + F],
                        )
                        (sq2h0 if half == 0 else sq2h1).append(sq)
                else:
                    nc.vector.scalar_tensor_tensor(
                        x2T[:, m, :], ps[:, :], bproj_sb[:, m : m + 1],
                        xt_sb[:, m, :], OP.add, OP.add,
                    )
                    for half, lst in ((0, sq2h0), (1, sq2h1)):
                        c0 = half * F
                        sq = sqp.tile([P, F], BF16, tag="sq", name="sq")
                        nc.vector.tensor_mul(
                            sq[:], x2T[:, m, c0 : c0 + F],
                            x2T[:, m, c0 : c0 + F],
                        )
                        lst.append(sq)

            sq2h0, sq2h1 = [], []
            emit_proj_m(0, spA.tile([P, 2 * F], F32, tag="sp", name="prps"))
            emit_proj_m(1, spA.tile([P, 2 * F], F32, tag="sp", name="prps"))
            emit_proj_m(2, opsw.tile([P, 2 * F], F32, tag="ow", name="prps"))
            emit_proj_m(3, opsw.tile([P, 2 * F], F32, tag="ow", name="prps"))
            emit_proj_m(4, spA.tile([P, 2 * F], F32, tag="sp", name="prps"))
            emit_proj_m(5, opsw.tile([P, 2 * F], F32, tag="ow", name="prps"))

            opsw.release()
            spA.release()
            # mlpw 3-wide + a separate 1-wide tail pool: fc2's last output
            # tile comes from the tail, so mlpw can release before the rep
            # boundary and the next rep's LN1 stats aren't gated on it.
            mlp_tail = tc.alloc_tile_pool(name="mlptl", bufs=1, space="PSUM")
            mlpw = tc.alloc_tile_pool(name="mlpw", bufs=3, space="PSUM")

            # ---- LN2 (squares were precomputed in the proj loop); h0 stats
            # on the tail pool, h1 on mlpw, so neither extends the release
            # chain that fc1's wave tiles wait on; norm-h0 sits between the
            # chains so fc1's first wave is never DVE-blocked ----
            s2h0 = emit_ln_stats(x2T, 0, mlp_tail, sq2h0)
            emit_ln_norm(x2T, h2T, 0, s2h0[2], s2h0[3])
            s2h1 = emit_ln_stats(x2T, 1, mlpw, sq2h1)
            emit_ln_norm(x2T, h2T, 1, s2h1[2], s2h1[3])

            # ---- fc1: m0..5 half-at-a-time (hides LN2-h1 latency), rest
            # full-width; two m's share one wide PSUM tile ----
            for c0 in (0, F):
                for mp in range(KD // 2):
                    ps = mlpw.tile([P, 2 * F], F32, tag="wd", name="f1ps")
                    for sub in (0, 1):
                        m = 2 * mp + sub
                        for kk in range(KD):
                            nc.tensor.matmul(
                                ps[:, sub * F : (sub + 1) * F],
                                f1_tiles[m][:, kk, :], h2T[:, kk, c0 : c0 + F],
                                start=(kk == 0), stop=(kk == KD - 1),
                            )
                    for sub in (0, 1):
                        m = 2 * mp + sub
                        nc.scalar.activation(
                            gT[:, m, c0 : c0 + F],
                            ps[:, sub * F : (sub + 1) * F],
                            AF.Gelu, bias=bfc1_sb[:, m : m + 1],
                        )
            for m in range(KD, MF1):
                wt = f1s.tile([P, KD, P], BF16, tag="f1", name="f1w")
                nc.sync.dma_start(out=wt[:], in_=wfc1[m])
                ps = mlpw.tile([P, 2 * F], F32, tag="wd", name="f1wd")
                for half in range(NHALF):
                    c0 = half * F
                    for kk in range(KD):
                        nc.tensor.matmul(
                            ps[:, c0 : c0 + F],
                            wt[:, kk, :], h2T[:, kk, c0 : c0 + F],
                            start=(kk == 0), stop=(kk == KD - 1),
                        )
                nc.scalar.activation(
                    gT[:, m, :], ps[:], AF.Gelu, bias=bfc1_sb[:, m : m + 1]
                )

            # Prewarm the sqrt table during the ACT-idle fc2 phase so the
            # next rep's LN1 Sqrt doesn't eat the table load.
            warm3 = stat.tile([1, 1], F32, tag="st", name="warm3")
            nc.scalar.activation(warm3[:], eps_sb[:], AF.Sqrt)

            # ---- fc2 + residual -> out (half-width output pieces) ----
            for m in range(KD):
                if m == 0:
                    w2 = w2_first
                else:
                    w2 = f2s.tile([P, MF1, P], BF16, tag="f2", name="f2w")
                    nc.sync.dma_start(out=w2[:], in_=wfc2[m])
                if m == KD - 1:
                    mlpw.release()
                    ps = mlp_tail.tile([P, 2 * F], F32, tag="wd", name="f2wd")
                else:
                    ps = mlpw.tile([P, 2 * F], F32, tag="wd", name="f2wd")
                for half in range(NHALF):
                    c0 = half * F
                    for kk in range(MF1):
                        nc.tensor.matmul(
                            ps[:, c0 : c0 + F],
                            w2[:, kk, :], gT[:, kk, c0 : c0 + F],
                            start=(kk == 0), stop=(kk == MF1 - 1),
                        )
                for half in range(NHALF):
                    c0 = half * F
                    yo = outp.tile([P, F], F32, tag="yo", name="yo")
                    nc.vector.scalar_tensor_tensor(
                        yo[:], ps[:, c0 : c0 + F], bfc2_sb[:, m : m + 1],
                        x2T[:, m, c0 : c0 + F], OP.add, OP.add,
                    )
                    nc.sync.dma_start(out=yt[:, m, c0 : c0 + F], in_=yo[:])

            g_pool.release()
            h2_pool.release()
            x2_pool.release()
            attn_pool.release()
            xt_pool.release()
            mlp_tail.release()

        f2s.release()
        f1s.release()
        wstream.release()
        outp.release()
        ptp.release()
        tmp.release()
        sqp.release()
        bcR.release()
        bcP.release()
        bc1.release()
        stat.release()
        const.release()

    nc.compile()
    return nc


def _retile_w(w_t, mtiles):
    """[out, in] weight -> [mtiles, P, in//P, P]: t[m,p,kk,o] = w[m*P+o, kk*P+p]."""
    out_dim, in_dim = w_t.shape
    a = w_t.reshape(mtiles, P, in_dim // P, P).transpose(0, 3, 2, 1)
    return np.ascontiguousarray(a)


def _rhs_tile(w_t):
    """[KD*P, in] weight -> [P, in//P, KD, P]: t[p,kk,m,o] = w[m*P+o, kk*P+p]."""
    a = w_t.reshape(KD, P, w_t.shape[1] // P, P).transpose(3, 2, 0, 1)
    return np.ascontiguousarray(a)


_NC_CACHE = {}


def _get_nc():
    if "nc" not in _NC_CACHE:
        _NC_CACHE["nc"] = build_program()
    return _NC_CACHE["nc"]


def prep_inputs(x, ln1_w, ln1_b, qkv_w, qkv_b, proj_w, proj_b,
                ln2_w, ln2_b, fc1_w, fc1_b, fc2_w, fc2_b):
    import ml_dtypes

    bf16 = np.dtype(ml_dtypes.bfloat16)
    f32 = lambda a: np.asarray(a, dtype=np.float32)
    x = f32(x)
    qkv_w, qkv_b = f32(qkv_w), f32(qkv_b)
    fc1_w, fc1_b = f32(fc1_w), f32(fc1_b)

    # fold LN gain/bias into the consuming layer
    wqkv_eff = qkv_w * f32(ln1_w)[None, :]
    bqkv_eff = qkv_b + qkv_w @ f32(ln1_b)
    wfc1_eff = fc1_w * f32(ln2_w)[None, :]
    bfc1_eff = fc1_b + fc1_w @ f32(ln2_b)

    # v bias is exact to fold into the proj bias: softmax weights sum to 1,
    # so attn_out = PV/denom + bv and proj(attn_out) = proj(PV/denom) +
    # proj_w @ bv + proj_b.
    bproj_eff = f32(proj_b) + f32(proj_w) @ bqkv_eff[2 * D :]

    shared = {
        "onesb": np.ones((P, 1), dtype=bf16),
        "wqk": _retile_w(wqkv_eff[: 2 * D], 2 * KD).astype(bf16),
        "wv": _rhs_tile(wqkv_eff[2 * D :]).astype(bf16),
        "bqk_pp": np.ascontiguousarray(
            bqkv_eff[: 2 * D].reshape(2 * KD, P).T
        ).astype(np.float32),
        "wqk_rsum": np.ascontiguousarray(
            -wqkv_eff[: 2 * D].sum(axis=1)[None, :]
        ).astype(bf16),
        "wv_rsum": np.ascontiguousarray(
            -wqkv_eff[2 * D :].sum(axis=1)[None, :]
        ).astype(bf16),
        "wproj": _rhs_tile(f32(proj_w)).astype(bf16),
        "bproj_pp": np.ascontiguousarray(
            bproj_eff.reshape(KD, P).T
        ).astype(np.float32),
        "wfc1": _retile_w(wfc1_eff, MF1).astype(bf16),
        "bfc1": np.ascontiguousarray(bfc1_eff.reshape(MF1, P).T),
        "wfc2": _retile_w(f32(fc2_w), KD).astype(bf16),
        "bfc2_pp": np.ascontiguousarray(
            f32(fc2_b).reshape(KD, P).T
        ).astype(np.float32),
    }
    in_maps = []
    for b in range(N_CORES):
        m = dict(shared)
        # xt[p, s, n] = x[b, n, s*P + p]
        m["xt"] = np.ascontiguousarray(
            x[b].reshape(NTOK, KD, P).transpose(2, 1, 0)
        ).astype(bf16)
        in_maps.append(m)
    return in_maps


def kernel(**inputs):
    nc = _get_nc()
    in_maps = prep_inputs(**inputs)
    res = run_bass_kernel_spmd(nc, in_maps, list(range(N_CORES)))
    outs = []
    for b in range(N_CORES):
        ytile = res.results[b]["yt"]  # [P, KD, NTOK]
        outs.append(ytile.transpose(2, 1, 0).reshape(NTOK, D))
    return np.stack(outs).astype(np.float32)


# revision 85
# speedup vs baseline: 1.4130x; 1.4130x over previous
"""Trainium2 Bass kernel for a pre-norm transformer block (B=8, N=1024, C=768).

Data-parallel over batch: each of the 8 NeuronCores runs the full block for
one batch element. Activations are feature-major ([feat, tok]) so matmuls
contract over the partition dim with no on-device transposes.

vs the f32r baseline:
  - bf16 matmul operands end-to-end (PSUM accumulation stays f32); rel err
    ~5e-3, well under the 2e-2 gate. Halves DMA and enables DVE 2x modes.
  - LN gain/bias folded into downstream weights/biases on the host. k and v
    chunks bypass the LN1 normalize entirely: W(x-mu)*rs is computed as
    (W@x - wsum (x) mu) * rs off RAW x with one rank-1 correction matmul
    per group (k bias cancels in softmax; v bias folds into the proj bias
    since softmax weights sum to 1), so the qkv stream starts as soon as
    the mean is known. Only the q chunks wait for the full normalize.
  - The LN stat chain is latency-tuned: DVE squares (bf16 2x), fused
    variance stt, mean broadcast issued before the sqrt/recip tail, subs
    before muls in the normalize. LN2's squares/stt ride the proj residual
    stream; LN2 stats run from the mlp PSUM pools so the fc1 wave tiles
    never extend the attention-pool release chain.
  - q biases ride the PSUM->SBUF move as ACT Identity ops with a
    per-partition bias AP; proj/fc2 biases fuse into the residual add as
    scalar_tensor_tensor on DVE. No ones-row bias matmuls remain.
  - All ACT table loads are prewarmed by dummy ops placed where ACT idles
    (exp before attention, sqrt behind the last exp and the last gelu), so
    no 1.3us table load lands on a critical chain.
  - Attention is software-pipelined: scores(kc) emit before PV(kc-1); one
    exp per head-PAIR per key-chunk ([128,1024] 2-bank PSUM tiles). Pairs
    run in order [5,0,1,2,3,4]; the 8 late q/k chunk groups run at
    attention start from the score-tile ring with DVE moves (ACT stays
    exp-only); proj contracts chunks in pair order and borrows all six
    PSUM tiles from the attention rings, so the pool release boundary
    never parks the PE behind the last pair's normalize.
  - fc1/fc2 weights stream once per rep; first fc1/fc2 loads are issued
    during attention. fc2's last output tile uses a separate 1-wide PSUM
    pool so the main MLP pool releases before the rep boundary and the
    next rep's LN1 stats start immediately.
"""

import os
import sys

import numpy as np

for _p in ("/opt/trn_rl_repo", "/root/.axon_site/_ro/trn_rl_repo"):
    if os.path.isdir(_p) and _p not in sys.path:
        sys.path.append(_p)

import concourse.bass as bass  # noqa: E402
import concourse.tile as tile  # noqa: E402
from concourse import bacc, mybir  # noqa: E402
from concourse.bass_utils import run_bass_kernel_spmd  # noqa: E402

F32 = mybir.dt.float32
BF16 = mybir.dt.bfloat16
AF = mybir.ActivationFunctionType
OP = mybir.AluOpType

P = 128
D = 768
KD = D // P          # 6 feature chunks over the 768 contraction dim
NTOK = 1024
F = 512              # token-half width (matmul free dim)
NHALF = NTOK // F    # 2
H = 12
DK = 64
DFF = 3072
MF1 = DFF // P       # 24
TC = NTOK // P       # 8 key/token chunks
EPS = 1e-5
N_CORES = 8

P_ORDER = (5, 0, 1, 2, 3, 4)  # attention pair order; proj contracts likewise


def build_program(reps=1):
    stage = int(os.environ.get("KERNEL_STAGE", "5"))
    nc = bacc.Bacc(
        "TRN2", target_bir_lowering=False, debug=False, num_devices=N_CORES
    )

    din = lambda name, shape, dt=BF16: nc.dram_tensor(
        name, shape, dt, kind="ExternalInput"
    ).ap()
    xt = din("xt", [P, KD, NTOK])
    onesb = din("onesb", [P, 1])
    wqk = din("wqk", [2 * KD, P, KD, P])
    wv = din("wv", [P, KD, KD, P])
    bqk_pp = din("bqk_pp", [P, 2 * KD], F32)     # q/k bias, [part, chunk]
    wqk_rsum = din("wqk_rsum", [1, 2 * D])       # NEGATED q/k row-sums
    wv_rsum = din("wv_rsum", [1, D])             # NEGATED v row-sums
    wproj = din("wproj", [P, KD, KD, P])
    bproj_pp = din("bproj_pp", [P, KD], F32)
    wfc1 = din("wfc1", [MF1, P, KD, P])
    bfc1 = din("bfc1", [P, MF1], F32)
    wfc2 = din("wfc2", [KD, P, MF1, P])
    bfc2_pp = din("bfc2_pp", [P, KD], F32)
    yt = nc.dram_tensor("yt", [P, KD, NTOK], F32, kind="ExternalOutput").ap()
    # DRAM bounce row for the [1,F] -> [tok_part, chunk] rstd transpose
    # (SBUF->SBUF partition-crossing APs don't balance; DRAM APs are free).
    rs_scr = nc.dram_tensor("rs_scr", [1, NTOK], F32, kind="Internal").ap()

    with tile.TileContext(nc) as tc:
        # PSUM pools are all per-phase (8-bank budget): qkv spp(3 wide),
        # attention spA(2 wide)+opsw(2 wide), MLP mlpw(4 wide).
        # ---- constant/global SBUF pools ----
        const = tc.alloc_tile_pool(name="const", bufs=1)
        stat = tc.alloc_tile_pool(name="stat", bufs=5)
        bc1 = tc.alloc_tile_pool(name="bc1", bufs=4)     # [1,F] bf16 casts
        bcP = tc.alloc_tile_pool(name="bcP", bufs=4)     # [P,F] bf16 bcasts
        bcR = tc.alloc_tile_pool(name="bcR", bufs=2)     # [DK,2F] f32 bcasts
        sqp = tc.alloc_tile_pool(name="sqp", bufs=12)
        tmp = tc.alloc_tile_pool(name="tmp", bufs=6)
        ptp = tc.alloc_tile_pool(name="ptp", bufs=4)
        outp = tc.alloc_tile_pool(name="outp", bufs=3)
        wstream = tc.alloc_tile_pool(name="wstream", bufs=4)
        f1s = tc.alloc_tile_pool(name="f1s", bufs=8)
        f2s = tc.alloc_tile_pool(name="f2s", bufs=2)

        eps_sb = const.tile([1, 1], F32)
        nc.vector.memset(eps_sb, EPS)
        onesrow = const.tile([1, NTOK], BF16)
        nc.vector.memset(onesrow, 1.0)
        onesb_sb = const.tile([P, 1], BF16, name="onesb_sb")
        nc.sync.dma_start(out=onesb_sb[:], in_=onesb[:])

        def load_const(ap_dram, shape=None, dt=None):
            t = const.tile(
                shape or list(ap_dram.shape), dt or ap_dram.dtype,
                name=ap_dram.name + "_sb",
            )
            nc.sync.dma_start(out=t[:], in_=ap_dram[:])
            return t

        # small biases up front; the two big weight consts (wv, wproj) are
        # deferred until after rep-0's xt DMA so LN1 isn't starved.
        bqk_sb = load_const(bqk_pp)
        wqk_rsum_sb = load_const(wqk_rsum)
        wv_rsum_sb = load_const(wv_rsum)
        bproj_sb = load_const(bproj_pp)
        bfc1_sb = load_const(bfc1)
        bfc2_sb = load_const(bfc2_pp)
        deferred = {}

        # ---- layernorm: stats via ones-matmuls (one wide PSUM tile:
        # sum in bank 0, sumsq in bank 1), 2-op normalize.
        # rstd = Exp(-0.5*Ln(var+eps)) -- stays in the nat_log_exp table ----
        def emit_ln_stats(src, half, pool, sqs=None, tag="wd"):
            cols = slice(half * F, (half + 1) * F)
            st_ps = pool.tile([P, 2 * F], F32, tag=tag, name="st_ps")
            # Squares on DVE (bf16 2x mode, ~0.27us each) so the sumsq pass
            # never waits on the ACT queue.
            if sqs is None:
                sqs = []
                for kk in range(KD):
                    sq = sqp.tile([P, F], BF16, tag="sq", name="sq")
                    nc.vector.tensor_mul(
                        sq[:], src[:, kk, cols], src[:, kk, cols]
                    )
                    sqs.append(sq)
            for kk in range(KD):
                nc.tensor.matmul(
                    st_ps[0:1, 0:F], onesb_sb[:], src[:, kk, cols],
                    start=(kk == 0), stop=(kk == KD - 1),
                )
            for kk in range(KD):
                nc.tensor.matmul(
                    st_ps[0:1, F : 2 * F], onesb_sb[:], sqs[kk][:],
                    start=(kk == 0), stop=(kk == KD - 1),
                )
            mu16 = bc1.tile([1, F], BF16, tag="b1", name="mu16")
            nc.vector.tensor_scalar_mul(mu16[:], st_ps[0:1, 0:F], 1.0 / D)
            # broadcast the mean immediately: the normalize subs only need
            # mu_b, so they overlap the variance/sqrt/recip tail
            mu_b = bcP.tile([P, F], BF16, tag="bP", name="mu_b")
            nc.gpsimd.partition_broadcast(mu_b[:], mu16[:])
            m2 = stat.tile([1, F], F32, tag="st", name="m2")
            nc.vector.tensor_mul(m2[:], mu16[:], mu16[:])
            e2 = stat.tile([1, F], F32, tag="st", name="e2")
            nc.vector.scalar_tensor_tensor(
                e2[:], st_ps[0:1, F : 2 * F], 1.0 / D, m2[:],
                OP.mult, OP.subtract,
            )
            sd = stat.tile([1, F], F32, tag="st", name="sd")
            nc.scalar.activation(sd[:], e2[:], AF.Sqrt, bias=eps_sb[0:1])
            rs16 = bc1.tile([1, F], BF16, tag="b1", name="rs16")
            with nc.allow_low_precision(
                reason="rstd feeds bf16 normalize muls; bf16 out is exact "
                       "enough (values are O(1))"
            ):
                nc.vector.reciprocal(rs16[:], sd[:])
            rs_b = bcP.tile([P, F], BF16, tag="bP", name="rs_b")
            nc.gpsimd.partition_broadcast(rs_b[:], rs16[:])
            rs = stat.tile([1, F], F32, tag="st", name="rs")
            nc.vector.reciprocal(rs[:], sd[:])
            return mu16, rs16, mu_b, rs_b, rs

        def emit_ln_norm(src, dst, half, mu_b, rs_b):
            # all subs first (they need only mu_b, broadcast early), then
            # the muls: the sub pass runs during the sqrt/recip tail.
            cols = slice(half * F, (half + 1) * F)
            t1s = []
            for kk in range(KD):
                t1 = tmp.tile([P, F], BF16, tag="t", name="t1")
                nc.vector.tensor_tensor(
                    t1[:], src[:, kk, cols], mu_b[:], OP.subtract
                )
                t1s.append(t1)
            for kk in range(KD):
                nc.vector.tensor_mul(dst[:, kk, cols], t1s[kk][:], rs_b[:])

        def emit_ln(src, dst, pool, sqs0=None):
            # stats for both halves first (h1's squares never queue behind
            # h0's normalize on DVE; h1's Sqrt lands before downstream ACT
            # ops so the table never thrashes), then the normalizes.
            s0 = emit_ln_stats(src, 0, pool, sqs0)
            s1 = emit_ln_stats(src, 1, pool)
            emit_ln_norm(src, dst, 0, s0[2], s0[3])
            emit_ln_norm(src, dst, 1, s1[2], s1[3])

        for _rep in range(reps):
            spp = tc.alloc_tile_pool(name="spp", bufs=4, space="PSUM")

            xt_pool = tc.alloc_tile_pool(name="xt", bufs=1)
            attn_pool = tc.alloc_tile_pool(name="attn", bufs=1)
            qk_pool = tc.alloc_tile_pool(name="qk", bufs=12)
            vaug_pool = tc.alloc_tile_pool(name="vaug", bufs=1)
            h_pool = tc.alloc_tile_pool(name="h", bufs=1)

            xt_sb = xt_pool.tile([P, KD, NTOK], BF16, name="xt_sb")
            hT = h_pool.tile([P, KD, NTOK], BF16, name="hT")
            attnT = attn_pool.tile([P, KD, NTOK], BF16, name="attnT")
            v_aug = vaug_pool.tile([P, TC, H, DK + 1], BF16, name="v_aug")

            qk_tiles = {}

            # ---- q/k chunk: 12 matmuls -> ACT Identity+bias move to bf16 ----
            def qk_chunk_ops(m):
                wt = wstream.tile([P, KD, P], BF16, tag="w", name="wt")
                nc.sync.dma_start(out=wt[:], in_=wqk[m])
                qkt = qk_pool.tile([P, NTOK], BF16, tag="qkt", name="qkt")
                qk_tiles[m] = qkt
                holder = {}

                def group(half):
                    if half == 0:
                        holder["ps"] = spp.tile(
                            [P, 2 * F], F32, tag="wd", name="qkps"
                        )
                    c0 = half * F
                    for kk in range(KD):
                        nc.tensor.matmul(
                            holder["ps"][:, c0 : c0 + F],
                            wt[:, kk, :], hT[:, kk, c0 : c0 + F],
                            start=(kk == 0), stop=(kk == KD - 1 and half == 1),
                        )

                def move():
                    nc.scalar.activation(
                        qkt[:, :], holder["ps"][:, :], AF.Identity,
                        bias=bqk_sb[:, m : m + 1],
                    )

                return group, move

            # ---- k chunk via LN bypass: W(x-mu)*rs computed as
            # (W@x - wsum (x) mu) * rs, consuming RAW x. The key-side bias
            # cancels in softmax, so this is exact -- and it frees the k/v
            # stream from the LN1 normalize chain entirely. ----
            def emit_k_bypass_group(m, mu16s):
                wt = wstream.tile([P, KD, P], BF16, tag="w", name="wt")
                nc.sync.dma_start(out=wt[:], in_=wqk[m])
                kt = qk_pool.tile([P, NTOK], BF16, tag="qkt", name="qkt")
                qk_tiles[m] = kt
                ps = spp.tile([P, 2 * F], F32, tag="wd", name="kps")
                for half in range(NHALF):
                    c0 = half * F
                    for kk in range(KD):
                        nc.tensor.matmul(
                            ps[:, c0 : c0 + F],
                            wt[:, kk, :], xt_sb[:, kk, c0 : c0 + F],
                            start=(kk == 0), stop=False,
                        )
                    nc.tensor.matmul(
                        ps[:, c0 : c0 + F],
                        wqk_rsum_sb[0:1, m * P : (m + 1) * P],
                        mu16s[half][:],
                        start=False, stop=True,
                    )
                return kt, ps

            def emit_k_bypass_move(kt_ps, rs_bs):
                kt, ps = kt_ps
                for half in range(NHALF):
                    c0 = half * F
                    nc.vector.tensor_mul(
                        kt[:, c0 : c0 + F], ps[:, c0 : c0 + F], rs_bs[half][:]
                    )

            # ---- v chunk (token-major), same bypass; the per-token rstd
            # rides the ACT Copy's scale port (v bias is folded into the
            # proj bias on the host: softmax weights sum to 1) ----
            def emit_v(t, mu16s, rs_tok):
                half = t * P // F
                trange = slice(t * P, (t + 1) * P)
                lt = slice(t * P - half * F, (t + 1) * P - half * F)
                ps = spp.tile([P, 2 * F], F32, tag="wd", name="vps")
                for c0, w in ((0, 512), (512, 256)):
                    for kk in range(KD):
                        nc.tensor.matmul(
                            ps[:, c0 : c0 + w],
                            xt_sb[:, kk, trange],
                            wv_sb[:, kk, c0 // P : (c0 + w) // P, :],
                            start=(kk == 0), stop=False,
                        )
                    nc.tensor.matmul(
                        ps[:, c0 : c0 + w],
                        mu16s[half][0:1, lt],
                        wv_rsum_sb[0:1, c0 : c0 + w],
                        start=False, stop=True,
                    )
                nc.scalar.activation(
                    v_aug[:, t, :, 0:DK],
                    ps[:, 0:D].rearrange("p (h d) -> p h d", d=DK),
                    AF.Copy,
                    scale=rs_tok[:, t : t + 1],
                )

            # DMA issue order feeds compute in need-order: xt h0 (LN1-h0),
            # pair-5 k weights, xt h1, then wv; wproj is deferred to the
            # attention phase.
            for kk in range(KD):
                nc.sync.dma_start(out=xt_sb[:, kk, 0:F], in_=xt[:, kk, 0:F])
            s0 = emit_ln_stats(xt_sb, 0, spp)
            for kk in range(KD):
                nc.sync.dma_start(
                    out=xt_sb[:, kk, F : 2 * F], in_=xt[:, kk, F : 2 * F]
                )
            if _rep == 0:
                deferred["wv_sb"] = load_const(wv)
            wv_sb = deferred["wv_sb"]
            nc.vector.memset(v_aug[:, :, :, DK : DK + 1], 1.0)

            s1 = emit_ln_stats(xt_sb, 1, spp)
            mu16s = (s0[0], s1[0])
            rs_bs = (s0[3], s1[3])

            # k and v stream off raw x while the normalize (needed only by
            # the q chunks and the interleaved pairs) completes on DVE. The
            # k-moves are emitted after the norms so the DVE queue serves
            # the q-chunk dependency (norm-h0) first.
            k5 = emit_k_bypass_group(11, mu16s)
            k0 = emit_k_bypass_group(6, mu16s)
            # transpose rstd to token-major [tok_part, chunk] for the v
            # moves. Emitted here (not right after stats) so the DRAM-bounce
            # DMAs, which wait on the rstd chain, never head-block the queue
            # entries for the k/q/prefetch weight loads.
            rs_tok = bc1.tile([P, TC], F32, tag="rt", name="rs_tok")
            for half, s in ((0, s0), (1, s1)):
                cols = slice(half * F, (half + 1) * F)
                nc.sync.dma_start(out=rs_scr[0:1, cols], in_=s[4][:])
                nc.sync.dma_start(
                    out=rs_tok[:, half * 4 : (half + 1) * 4],
                    in_=rs_scr[0:1, cols].rearrange("a (c p) -> (a p) c", p=P),
                )
            emit_v(0, mu16s, rs_tok)
            emit_v(1, mu16s, rs_tok)
            emit_ln_norm(xt_sb, hT, 0, s0[2], s0[3])
            emit_ln_norm(xt_sb, hT, 1, s1[2], s1[3])
            emit_k_bypass_move(k5, rs_bs)
            emit_k_bypass_move(k0, rs_bs)
            for t in range(2, TC):
                emit_v(t, mu16s, rs_tok)

            g_q5, mv_q5 = qk_chunk_ops(5)
            g_q5(0); g_q5(1); mv_q5()
            g_q0, mv_q0 = qk_chunk_ops(0)
            g_q0(0); g_q0(1); mv_q0()

            # Prewarm the exp table behind the last Copy move; the load
            # overlaps the first score matmuls instead of stalling the
            # mid-qkv move pipeline.
            warm1 = stat.tile([1, 1], F32, tag="st", name="warm1")
            nc.scalar.activation(warm1[:], eps_sb[:], AF.Exp)

            # Prefetch the remaining q/k chunk weights through the f1s pool
            # (idle until fc1; same tile shape, and the ring antideps line
            # up: fc1's first tiles land on slots whose attention readers
            # finish mid-attention).
            INTER = {5: (1, 7), 0: (2, 8), 1: (3, 9), 2: (4, 10)}
            wts = {}
            for w in P_ORDER[:4]:
                for m in INTER[w]:
                    wt = f1s.tile([P, KD, P], BF16, tag="f1", name="wqk_pre")
                    nc.sync.dma_start(out=wt[:], in_=wqk[m])
                    wts[m] = wt

            if _rep == 0:
                deferred["wproj_sb"] = load_const(wproj)
            wproj_sb = deferred["wproj_sb"]

            # ---- attention: double-buffered score tiles AND PV accumulators
            # so PE never couples to the ACT exp backlog ----
            spp.release()
            spA = tc.alloc_tile_pool(name="spA", bufs=2, space="PSUM")
            opsw = tc.alloc_tile_pool(name="opsw", bufs=2, space="PSUM")
            pranges = (slice(0, DK), slice(DK, P))

            # One global software pipeline over all (pair, half, kc) units:
            # S+exp for unit i, then PV for unit i-1 — no per-half drain.
            state = {}  # (j, half) -> dict(o_ps=..., pts={kc: pt})

            def emit_s_exp(j, half, kc):
                st = state.setdefault((j, half), {"pts": {}})
                if kc == 0:
                    st["o_ps"] = opsw.tile([P, 2 * F], F32, tag="ow",
                                           name="o_ps")
                q_t = qk_tiles[j]
                k_t = qk_tiles[KD + j]
                cols = slice(half * F, (half + 1) * F)
                sp = spA.tile([P, 2 * F], F32, tag="sp", name="sp")
                for hi in (0, 1):
                    pr = pranges[hi]
                    nc.tensor.matmul(
                        sp[:, hi * F : (hi + 1) * F],
                        k_t[pr, kc * P : (kc + 1) * P],
                        q_t[pr, cols],
                        start=True, stop=True,
                    )
                pt = ptp.tile([P, 2 * F], BF16, tag="pt", name="pt")
                nc.scalar.activation(
                    pt[:], sp[:], AF.Exp, scale=float(DK) ** -0.5
                )
                st["pts"][kc] = pt

            def emit_pv(j, half, kc):
                st = state[(j, half)]
                o_ps = st["o_ps"]
                pt = st["pts"].pop(kc)
                for hi in (0, 1):
                    nc.tensor.matmul(
                        o_ps[0 : DK + 1, hi * F : (hi + 1) * F],
                        v_aug[:, kc, 2 * j + hi, :],
                        pt[:, hi * F : (hi + 1) * F],
                        start=(kc == 0), stop=(kc == TC - 1),
                    )
                if kc == TC - 1:
                    # per-hi pipelined normalize: both recips (DVE), then
                    # both bcasts (Pool), then both muls (DVE) -- the in-order
                    # engine queues overlap across the two heads, halving the
                    # serial tail before the PSUM pool boundary.
                    cols = slice(half * F, (half + 1) * F)
                    recs, rbs = [], []
                    for hi in (0, 1):
                        rec = stat.tile([1, F], F32, tag="st", name="rec")
                        nc.vector.reciprocal(
                            rec[:], o_ps[DK : DK + 1, hi * F : (hi + 1) * F]
                        )
                        recs.append(rec)
                    for hi in (0, 1):
                        rec_b = bcR.tile([DK, F], F32, tag="bR", name="rec_b")
                        nc.gpsimd.partition_broadcast(rec_b[:], recs[hi][:])
                        rbs.append(rec_b)
                    for hi in (0, 1):
                        nc.vector.tensor_mul(
                            attnT[pranges[hi], j, cols],
                            o_ps[0:DK, hi * F : (hi + 1) * F],
                            rbs[hi][:],
                        )

            # Interleaved q/k chunk: 12 matmuls into a spA-ring tile fill the
            # PE while ACT works down the exp backlog; the PSUM->SBUF move
            # runs on DVE (idle during attention) so ACT never sees it.
            def emit_qk_inter(m):
                qkt = qk_pool.tile([P, NTOK], BF16, tag="qkt", name="qkt")
                qk_tiles[m] = qkt
                ps = spA.tile([P, 2 * F], F32, tag="sp", name="qkps")
                for half in range(NHALF):
                    c0 = half * F
                    for kk in range(KD):
                        nc.tensor.matmul(
                            ps[:, c0 : c0 + F],
                            wts[m][:, kk, :], hT[:, kk, c0 : c0 + F],
                            start=(kk == 0), stop=(kk == KD - 1 and half == 1),
                        )
                nc.vector.tensor_scalar_add(
                    qkt[:, :], ps[:, :], bqk_sb[:, m : m + 1]
                )

            units = [(j, half, kc)
                     for j in P_ORDER for half in range(NHALF)
                     for kc in range(TC)]
            # Emit the remaining chunks as a block here (not interleaved into
            # the unit stream: insertions starve ACT of exp backlog). The spA
            # ring + DVE moves keep them off the attention ACT queue.
            sched = {}
            for w in P_ORDER[:4]:
                for m in INTER[w]:
                    emit_qk_inter(m)

            # fc1/fc2 first-weight DMAs: emitted now (all readers of the f1s
            # slots they rotate into are recorded), so the transfers run
            # during attention and the MLP start is never DMA-paced.
            f1_tiles = []
            for m in range(KD):
                wt = f1s.tile([P, KD, P], BF16, tag="f1", name="f1w")
                nc.sync.dma_start(out=wt[:], in_=wfc1[m])
                f1_tiles.append(wt)
            w2_first = f2s.tile([P, MF1, P], BF16, tag="f2", name="f2w")
            nc.sync.dma_start(out=w2_first[:], in_=wfc2[0])
            SKEW = 1  # PE stays 1 S+exp unit ahead of the PV stream
            for i, u in enumerate(units):
                emit_s_exp(*u)
                if i >= SKEW:
                    emit_pv(*units[i - SKEW])
                if i in sched:
                    emit_qk_inter(sched[i])
            for u in units[-SKEW:]:
                emit_pv(*u)

            # Prewarm the sqrt table for LN2 behind the last exp; the load
            # overlaps proj's matmuls instead of the LN2 chain.
            warm2 = stat.tile([1, 1], F32, tag="st", name="warm2")
            nc.scalar.activation(warm2[:], eps_sb[:], AF.Sqrt)

            h_pool.release()
            vaug_pool.release()
            qk_pool.release()

            # ---- proj + residual -> x2T; contraction follows P_ORDER so the
            # last pair's normalize overlaps the first 5 chunks' matmuls.
            # m0..m3 borrow the attention pools' PSUM tiles (their per-buffer
            # antideps resolve pair-by-pair), so the pool release boundary
            # never parks the PE behind the last pair's normalize chain ----
            x2_pool = tc.alloc_tile_pool(name="x2", bufs=1, side="right")
            h2_pool = tc.alloc_tile_pool(name="h2", bufs=1, side="right")
            g_pool = tc.alloc_tile_pool(name="g", bufs=1, side="right")
            x2T = x2_pool.tile([P, KD, NTOK], BF16, name="x2T")
            h2T = h2_pool.tile([P, KD, NTOK], BF16, name="h2T")
            gT = g_pool.tile([P, MF1, NTOK], BF16, name="gT")

            def emit_proj_m(m, ps):
                for half in range(NHALF):
                    c0 = half * F
                    for i, kk in enumerate(P_ORDER):
                        nc.tensor.matmul(
                            ps[:, c0 : c0 + F],
                            wproj_sb[:, kk, m, :],
                            attnT[:, kk, c0 : c0 + F],
                            start=(i == 0), stop=(i == KD - 1),
                        )
                # LN2 squares for this chunk ride right behind the stt on
                # DVE, so LN2 stats never wait on a square backlog. The last
                # chunk's stt is split per half so its h0 lands sooner (it
                # gates the LN2-h0 sum pass).
                if m == KD - 1:
                    for half in range(NHALF):
                        c0 = half * F
                        nc.vector.scalar_tensor_tensor(
                            x2T[:, m, c0 : c0 + F], ps[:, c0 : c0 + F],
                            bproj_sb[:, m : m + 1],
                            xt_sb[:, m, c0 : c0 + F], OP.add, OP.add,
                        )
                        sq = sqp.tile([P, F], BF16, tag="sq", name="sq")
                        nc.vector.tensor_mul(
                            sq[:], x2T[:, m, c0 : c0 + F],
                            x2T[:, m, c0 : c0 + F],
                        )
                        (sq2h0 if half == 0 else sq2h1).append(sq)
                else:
                    nc.vector.scalar_tensor_tensor(
                        x2T[:, m, :], ps[:, :], bproj_sb[:, m : m + 1],
                        xt_sb[:, m, :], OP.add, OP.add,
                    )
                    for half, lst in ((0, sq2h0), (1, sq2h1)):
                        c0 = half * F
                        sq = sqp.tile([P, F], BF16, tag="sq", name="sq")
                        nc.vector.tensor_mul(
                            sq[:], x2T[:, m, c0 : c0 + F],
                            x2T[:, m, c0 : c0 + F],
                        )
                        lst.append(sq)

            sq2h0, sq2h1 = [], []
            emit_proj_m(0, spA.tile([P, 2 * F], F32, tag="sp", name="prps"))
            emit_proj_m(1, spA.tile([P, 2 * F], F32, tag="sp", name="prps"))
            emit_proj_m(2, opsw.tile([P, 2 * F], F32, tag="ow", name="prps"))
            emit_proj_m(3, opsw.tile([P, 2 * F], F32, tag="ow", name="prps"))
            emit_proj_m(4, spA.tile([P, 2 * F], F32, tag="sp", name="prps"))
            emit_proj_m(5, opsw.tile([P, 2 * F], F32, tag="ow", name="prps"))

            opsw.release()
            spA.release()
            # mlpw 3-wide + a separate 1-wide tail pool: fc2's last output
            # tile comes from the tail, so mlpw can release before the rep
            # boundary and the next rep's LN1 stats aren't gated on it.
            mlp_tail = tc.alloc_tile_pool(name="mlptl", bufs=1, space="PSUM")
            mlpw = tc.alloc_tile_pool(name="mlpw", bufs=3, space="PSUM")

            # ---- LN2 (squares were precomputed in the proj loop); h0 stats
            # on the tail pool, h1 on mlpw, so neither extends the release
            # chain that fc1's wave tiles wait on; norm-h0 sits between the
            # chains so fc1's first wave is never DVE-blocked ----
            s2h0 = emit_ln_stats(x2T, 0, mlp_tail, sq2h0)
            emit_ln_norm(x2T, h2T, 0, s2h0[2], s2h0[3])
            s2h1 = emit_ln_stats(x2T, 1, mlpw, sq2h1)
            emit_ln_norm(x2T, h2T, 1, s2h1[2], s2h1[3])

            # ---- fc1: m0..5 half-at-a-time (hides LN2-h1 latency), rest
            # full-width; two m's share one wide PSUM tile ----
            for c0 in (0, F):
                for mp in range(KD // 2):
                    ps = mlpw.tile([P, 2 * F], F32, tag="wd", name="f1ps")
                    for sub in (0, 1):
                        m = 2 * mp + sub
                        for kk in range(KD):
                            nc.tensor.matmul(
                                ps[:, sub * F : (sub + 1) * F],
                                f1_tiles[m][:, kk, :], h2T[:, kk, c0 : c0 + F],
                                start=(kk == 0), stop=(kk == KD - 1),
                            )
                    for sub in (0, 1):
                        m = 2 * mp + sub
                        nc.scalar.activation(
                            gT[:, m, c0 : c0 + F],
                            ps[:, sub * F : (sub + 1) * F],
                            AF.Gelu, bias=bfc1_sb[:, m : m + 1],
                        )
            for m in range(KD, MF1):
                wt = f1s.tile([P, KD, P], BF16, tag="f1", name="f1w")
                nc.sync.dma_start(out=wt[:], in_=wfc1[m])
                ps = mlpw.tile([P, 2 * F], F32, tag="wd", name="f1wd")
                for half in range(NHALF):
                    c0 = half * F
                    for kk in range(KD):
                        nc.tensor.matmul(
                            ps[:, c0 : c0 + F],
                            wt[:, kk, :], h2T[:, kk, c0 : c0 + F],
                            start=(kk == 0), stop=(kk == KD - 1),
                        )
                nc.scalar.activation(
                    gT[:, m, :], ps[:], AF.Gelu, bias=bfc1_sb[:, m : m + 1]
                )

            # Prewarm the sqrt table during the ACT-idle fc2 phase so the
            # next rep's LN1 Sqrt doesn't eat the table load.
            warm3 = stat.tile([1, 1], F32, tag="st", name="warm3")
            nc.scalar.activation(warm3[:], eps_sb[:], AF.Sqrt)

            # ---- fc2 + residual -> out (half-width output pieces) ----
            for m in range(KD):
                if m == 0:
                    w2 = w2_first
                else:
                    w2 = f2s.tile([P, MF1, P], BF16, tag="f2", name="f2w")
                    nc.sync.dma_start(out=w2[:], in_=wfc2[m])
                if m == KD - 1:
                    mlpw.release()
                    ps = mlp_tail.tile([P, 2 * F], F32, tag="wd", name="f2wd")
                else:
                    ps = mlpw.tile([P, 2 * F], F32, tag="wd", name="f2wd")
                for half in range(NHALF):
                    c0 = half * F
                    for kk in range(MF1):
                        nc.tensor.matmul(
                            ps[:, c0 : c0 + F],
                            w2[:, kk, :], gT[:, kk, c0 : c0 + F],
                            start=(kk == 0), stop=(kk == MF1 - 1),
                        )
                for half in range(NHALF):
                    c0 = half * F
                    yo = outp.tile([P, F], F32, tag="yo", name="yo")
                    nc.vector.scalar_tensor_tensor(
                        yo[:], ps[:, c0 : c0 + F], bfc2_sb[:, m : m + 1],
                        x2T[:, m, c0 : c0 + F], OP.add, OP.add,
                    )
                    nc.sync.dma_start(out=yt[:, m, c0 : c0 + F], in_=yo[:])

            g_pool.release()
            h2_pool.release()
            x2_pool.release()
            attn_pool.release()
            xt_pool.release()
            mlp_tail.release()

        f2s.release()
        f1s.release()
        wstream.release()
        outp.release()
        ptp.release()
        tmp.release()
        sqp.release()
        bcR.release()
        bcP.release()
        bc1.release()
        stat.release()
        const.release()

    nc.compile()
    return nc


def _retile_w(w_t, mtiles):
    """[out, in] weight -> [mtiles, P, in//P, P]: t[m,p,kk,o] = w[m*P+o, kk*P+p]."""
    out_dim, in_dim = w_t.shape
    a = w_t.reshape(mtiles, P, in_dim // P, P).transpose(0, 3, 2, 1)
    return np.ascontiguousarray(a)


def _rhs_tile(w_t):
    """[KD*P, in] weight -> [P, in//P, KD, P]: t[p,kk,m,o] = w[m*P+o, kk*P+p]."""
    a = w_t.reshape(KD, P, w_t.shape[1] // P, P).transpose(3, 2, 0, 1)
    return np.ascontiguousarray(a)


_NC_CACHE = {}


def _get_nc():
    if "nc" not in _NC_CACHE:
        _NC_CACHE["nc"] = build_program()
    return _NC_CACHE["nc"]


def prep_inputs(x, ln1_w, ln1_b, qkv_w, qkv_b, proj_w, proj_b,
                ln2_w, ln2_b, fc1_w, fc1_b, fc2_w, fc2_b):
    import ml_dtypes

    bf16 = np.dtype(ml_dtypes.bfloat16)
    f32 = lambda a: np.asarray(a, dtype=np.float32)
    x = f32(x)
    qkv_w, qkv_b = f32(qkv_w), f32(qkv_b)
    fc1_w, fc1_b = f32(fc1_w), f32(fc1_b)

    # fold LN gain/bias into the consuming layer
    wqkv_eff = qkv_w * f32(ln1_w)[None, :]
    bqkv_eff = qkv_b + qkv_w @ f32(ln1_b)
    wfc1_eff = fc1_w * f32(ln2_w)[None, :]
    bfc1_eff = fc1_b + fc1_w @ f32(ln2_b)

    # v bias is exact to fold into the proj bias: softmax weights sum to 1,
    # so attn_out = PV/denom + bv and proj(attn_out) = proj(PV/denom) +
    # proj_w @ bv + proj_b.
    bproj_eff = f32(proj_b) + f32(proj_w) @ bqkv_eff[2 * D :]

    shared = {
        "onesb": np.ones((P, 1), dtype=bf16),
        "wqk": _retile_w(wqkv_eff[: 2 * D], 2 * KD).astype(bf16),
        "wv": _rhs_tile(wqkv_eff[2 * D :]).astype(bf16),
        "bqk_pp": np.ascontiguousarray(
            bqkv_eff[: 2 * D].reshape(2 * KD, P).T
        ).astype(np.float32),
        "wqk_rsum": np.ascontiguousarray(
            -wqkv_eff[: 2 * D].sum(axis=1)[None, :]
        ).astype(bf16),
        "wv_rsum": np.ascontiguousarray(
            -wqkv_eff[2 * D :].sum(axis=1)[None, :]
        ).astype(bf16),
        "wproj": _rhs_tile(f32(proj_w)).astype(bf16),
        "bproj_pp": np.ascontiguousarray(
            bproj_eff.reshape(KD, P).T
        ).astype(np.float32),
        "wfc1": _retile_w(wfc1_eff, MF1).astype(bf16),
        "bfc1": np.ascontiguousarray(bfc1_eff.reshape(MF1, P).T),
        "wfc2": _retile_w(f32(fc2_w), KD).astype(bf16),
        "bfc2_pp": np.ascontiguousarray(
            f32(fc2_b).reshape(KD, P).T
        ).astype(np.float32),
    }
    in_maps = []
    for b in range(N_CORES):
        m = dict(shared)
        # xt[p, s, n] = x[b, n, s*P + p]
        m["xt"] = np.ascontiguousarray(
            x[b].reshape(NTOK, KD, P).transpose(2, 1, 0)
        ).astype(bf16)
        in_maps.append(m)
    return in_maps


def kernel(**inputs):
    nc = _get_nc()
    in_maps = prep_inputs(**inputs)
    res = run_bass_kernel_spmd(nc, in_maps, list(range(N_CORES)))
    outs = []
    for b in range(N_CORES):
        ytile = res.results[b]["yt"]  # [P, KD, NTOK]
        outs.append(ytile.transpose(2, 1, 0).reshape(NTOK, D))
    return np.stack(outs).astype(np.float32)


# revision 94
# speedup vs baseline: 1.5594x; 1.1035x over previous
"""Trainium2 Bass kernel for a pre-norm transformer block (B=8, N=1024, C=768).

Data-parallel over batch: each of the 8 NeuronCores runs the full block for
one batch element. Activations are feature-major ([feat, tok]) so matmuls
contract over the partition dim with no on-device transposes.

vs the f32r baseline:
  - bf16 matmul operands end-to-end (PSUM accumulation stays f32); rel err
    ~5e-3, well under the 2e-2 gate. Halves DMA and enables DVE 2x modes.
  - LN gain/bias folded into downstream weights/biases on the host. k and v
    chunks bypass the LN1 normalize entirely: W(x-mu)*rs is computed as
    (W@x - wsum (x) mu) * rs off RAW x with one rank-1 correction matmul
    per group (k bias cancels in softmax; v bias folds into the proj bias
    since softmax weights sum to 1), so the qkv stream starts as soon as
    the mean is known. Only the q chunks wait for the full normalize.
  - The LN stat chain is latency-tuned: DVE squares (bf16 2x), fused
    variance stt, mean broadcast issued before the sqrt/recip tail, subs
    before muls in the normalize. LN2's squares/stt ride the proj residual
    stream; LN2 stats run from the mlp PSUM pools so the fc1 wave tiles
    never extend the attention-pool release chain.
  - q biases ride the PSUM->SBUF move as ACT Identity ops with a
    per-partition bias AP; proj/fc2 biases fuse into the residual add as
    scalar_tensor_tensor on DVE. No ones-row bias matmuls remain.
  - All ACT table loads are prewarmed by dummy ops placed where ACT idles
    (exp before attention, sqrt behind the last exp and the last gelu), so
    no 1.3us table load lands on a critical chain.
  - Attention is software-pipelined: scores(kc) emit before PV(kc-1); one
    exp per head-PAIR per key-chunk ([128,1024] 2-bank PSUM tiles). Pairs
    run in order [5,0,1,2,3,4]; the 8 late q/k chunk groups run at
    attention start from the score-tile ring with DVE moves (ACT stays
    exp-only); proj contracts chunks in pair order and borrows all six
    PSUM tiles from the attention rings, so the pool release boundary
    never parks the PE behind the last pair's normalize.
  - fc1/fc2 weights stream once per rep; first fc1/fc2 loads are issued
    during attention. fc2's last output tile uses a separate 1-wide PSUM
    pool so the main MLP pool releases before the rep boundary and the
    next rep's LN1 stats start immediately.
"""

import os
import sys

import numpy as np

for _p in ("/opt/trn_rl_repo", "/root/.axon_site/_ro/trn_rl_repo"):
    if os.path.isdir(_p) and _p not in sys.path:
        sys.path.append(_p)

import concourse.bass as bass  # noqa: E402
import concourse.tile as tile  # noqa: E402
from concourse import bacc, mybir  # noqa: E402
from concourse.bass_utils import run_bass_kernel_spmd  # noqa: E402

F32 = mybir.dt.float32
BF16 = mybir.dt.bfloat16
AF = mybir.ActivationFunctionType
OP = mybir.AluOpType

P = 128
D = 768
KD = D // P          # 6 feature chunks over the 768 contraction dim
NTOK = 1024
F = 512              # token-half width (matmul free dim)
NHALF = NTOK // F    # 2
H = 12
DK = 64
DFF = 3072
MF1 = DFF // P       # 24
TC = NTOK // P       # 8 key/token chunks
EPS = 1e-5
N_CORES = 8

P_ORDER = (5, 0, 1, 2, 3, 4)  # attention pair order; proj contracts likewise


def build_program(reps=1):
    stage = int(os.environ.get("KERNEL_STAGE", "5"))
    nc = bacc.Bacc(
        "TRN2", target_bir_lowering=False, debug=False, num_devices=N_CORES
    )

    din = lambda name, shape, dt=BF16: nc.dram_tensor(
        name, shape, dt, kind="ExternalInput"
    ).ap()
    xt = din("xt", [P, KD, NTOK])
    onesb = din("onesb", [P, 1])
    wqk = din("wqk", [2 * KD, P, KD, P])
    wv = din("wv", [P, KD, KD, P])
    bqk_pp = din("bqk_pp", [P, 2 * KD], F32)     # q/k bias, [part, chunk]
    wqk_rsum = din("wqk_rsum", [1, 2 * D])       # NEGATED q/k row-sums
    wv_rsum = din("wv_rsum", [1, D])             # NEGATED v row-sums
    wproj = din("wproj", [P, KD, KD, P])
    bproj_pp = din("bproj_pp", [P, KD], F32)
    wfc1 = din("wfc1", [MF1, P, KD, P])
    bfc1 = din("bfc1", [P, MF1], F32)
    wfc2 = din("wfc2", [KD, P, MF1, P])
    bfc2_pp = din("bfc2_pp", [P, KD], F32)
    yt = nc.dram_tensor("yt", [P, KD, NTOK], F32, kind="ExternalOutput").ap()
    # DRAM bounce row for the [1,F] -> [tok_part, chunk] rstd transpose
    # (SBUF->SBUF partition-crossing APs don't balance; DRAM APs are free).
    rs_scr = nc.dram_tensor("rs_scr", [1, NTOK], F32, kind="Internal").ap()

    with tile.TileContext(nc) as tc:
        # PSUM pools are all per-phase (8-bank budget): qkv spp(3 wide),
        # attention spA(2 wide)+opsw(2 wide), MLP mlpw(4 wide).
        # ---- constant/global SBUF pools ----
        const = tc.alloc_tile_pool(name="const", bufs=1)
        stat = tc.alloc_tile_pool(name="stat", bufs=5)
        bc1 = tc.alloc_tile_pool(name="bc1", bufs=4)     # [1,F] bf16 casts
        bcP = tc.alloc_tile_pool(name="bcP", bufs=4)     # [P,F] bf16 bcasts
        bcR = tc.alloc_tile_pool(name="bcR", bufs=2)     # [DK,2F] f32 bcasts
        sqp = tc.alloc_tile_pool(name="sqp", bufs=12)
        tmp = tc.alloc_tile_pool(name="tmp", bufs=6)
        ptp = tc.alloc_tile_pool(name="ptp", bufs=4)
        outp = tc.alloc_tile_pool(name="outp", bufs=3)
        wstream = tc.alloc_tile_pool(name="wstream", bufs=4)
        f1s = tc.alloc_tile_pool(name="f1s", bufs=8)
        f2s = tc.alloc_tile_pool(name="f2s", bufs=2)

        eps_sb = const.tile([1, 1], F32)
        nc.vector.memset(eps_sb, EPS)
        onesb_sb = const.tile([P, 1], BF16, name="onesb_sb")
        nc.sync.dma_start(out=onesb_sb[:], in_=onesb[:])

        def load_const(ap_dram, shape=None, dt=None):
            t = const.tile(
                shape or list(ap_dram.shape), dt or ap_dram.dtype,
                name=ap_dram.name + "_sb",
            )
            nc.sync.dma_start(out=t[:], in_=ap_dram[:])
            return t

        # small biases up front; the two big weight consts (wv, wproj) are
        # deferred until after rep-0's xt DMA so LN1 isn't starved.
        bqk_sb = load_const(bqk_pp)
        wqk_rsum_sb = load_const(wqk_rsum)
        wv_rsum_sb = load_const(wv_rsum)
        bproj_sb = load_const(bproj_pp)
        bfc1_sb = load_const(bfc1)
        bfc2_sb = load_const(bfc2_pp)
        deferred = {}

        # ---- layernorm: stats via ones-matmuls (one wide PSUM tile:
        # sum in bank 0, sumsq in bank 1), 2-op normalize.
        # rstd = Exp(-0.5*Ln(var+eps)) -- stays in the nat_log_exp table ----
        def emit_ln_stats(src, half, pool, sqs=None, tag="wd"):
            cols = slice(half * F, (half + 1) * F)
            st_ps = pool.tile([P, 2 * F], F32, tag=tag, name="st_ps")
            # Squares on DVE (bf16 2x mode, ~0.27us each) so the sumsq pass
            # never waits on the ACT queue.
            if sqs is None:
                sqs = []
                for kk in range(KD):
                    sq = sqp.tile([P, F], BF16, tag="sq", name="sq")
                    nc.vector.tensor_mul(
                        sq[:], src[:, kk, cols], src[:, kk, cols]
                    )
                    sqs.append(sq)
            for kk in range(KD):
                nc.tensor.matmul(
                    st_ps[0:1, 0:F], onesb_sb[:], src[:, kk, cols],
                    start=(kk == 0), stop=(kk == KD - 1),
                )
            for kk in range(KD):
                nc.tensor.matmul(
                    st_ps[0:1, F : 2 * F], onesb_sb[:], sqs[kk][:],
                    start=(kk == 0), stop=(kk == KD - 1),
                )
            mu16 = bc1.tile([1, F], BF16, tag="b1", name="mu16")
            nc.vector.tensor_scalar_mul(mu16[:], st_ps[0:1, 0:F], 1.0 / D)
            # broadcast the mean immediately: the normalize subs only need
            # mu_b, so they overlap the variance/sqrt/recip tail
            mu_b = bcP.tile([P, F], BF16, tag="bP", name="mu_b")
            nc.gpsimd.partition_broadcast(mu_b[:], mu16[:])
            m2 = stat.tile([1, F], F32, tag="st", name="m2")
            nc.vector.tensor_mul(m2[:], mu16[:], mu16[:])
            e2 = stat.tile([1, F], F32, tag="st", name="e2")
            nc.vector.scalar_tensor_tensor(
                e2[:], st_ps[0:1, F : 2 * F], 1.0 / D, m2[:],
                OP.mult, OP.subtract,
            )
            sd = stat.tile([1, F], F32, tag="st", name="sd")
            nc.scalar.activation(sd[:], e2[:], AF.Sqrt, bias=eps_sb[0:1])
            rs16 = bc1.tile([1, F], BF16, tag="b1", name="rs16")
            with nc.allow_low_precision(
                reason="rstd feeds bf16 normalize muls; bf16 out is exact "
                       "enough (values are O(1))"
            ):
                nc.vector.reciprocal(rs16[:], sd[:])
            rs_b = bcP.tile([P, F], BF16, tag="bP", name="rs_b")
            nc.gpsimd.partition_broadcast(rs_b[:], rs16[:])
            rs = stat.tile([1, F], F32, tag="st", name="rs")
            nc.vector.reciprocal(rs[:], sd[:])
            return mu16, rs16, mu_b, rs_b, rs

        def emit_ln_norm(src, dst, half, mu_b, rs_b):
            # all subs first (they need only mu_b, broadcast early), then
            # the muls: the sub pass runs during the sqrt/recip tail.
            cols = slice(half * F, (half + 1) * F)
            t1s = []
            for kk in range(KD):
                t1 = tmp.tile([P, F], BF16, tag="t", name="t1")
                nc.vector.tensor_tensor(
                    t1[:], src[:, kk, cols], mu_b[:], OP.subtract
                )
                t1s.append(t1)
            for kk in range(KD):
                nc.vector.tensor_mul(dst[:, kk, cols], t1s[kk][:], rs_b[:])

        def emit_ln(src, dst, pool, sqs0=None):
            # stats for both halves first (h1's squares never queue behind
            # h0's normalize on DVE; h1's Sqrt lands before downstream ACT
            # ops so the table never thrashes), then the normalizes.
            s0 = emit_ln_stats(src, 0, pool, sqs0)
            s1 = emit_ln_stats(src, 1, pool)
            emit_ln_norm(src, dst, 0, s0[2], s0[3])
            emit_ln_norm(src, dst, 1, s1[2], s1[3])

        for _rep in range(reps):
            spp = tc.alloc_tile_pool(name="spp", bufs=4, space="PSUM")

            xt_pool = tc.alloc_tile_pool(name="xt", bufs=1)
            attn_pool = tc.alloc_tile_pool(name="attn", bufs=1)
            qk_pool = tc.alloc_tile_pool(name="qk", bufs=12)
            vaug_pool = tc.alloc_tile_pool(name="vaug", bufs=1)
            h_pool = tc.alloc_tile_pool(name="h", bufs=1)

            xt_sb = xt_pool.tile([P, KD, NTOK], BF16, name="xt_sb")
            hT = h_pool.tile([P, KD, NTOK], BF16, name="hT")
            attnT = attn_pool.tile([P, KD, NTOK], BF16, name="attnT")
            v_aug = vaug_pool.tile([P, TC, H, DK + 1], BF16, name="v_aug")

            qk_tiles = {}

            # ---- q/k chunk: 12 matmuls -> ACT Identity+bias move to bf16 ----
            def qk_chunk_ops(m):
                wt = wstream.tile([P, KD, P], BF16, tag="w", name="wt")
                nc.sync.dma_start(out=wt[:], in_=wqk[m])
                qkt = qk_pool.tile([P, NTOK], BF16, tag="qkt", name="qkt")
                qk_tiles[m] = qkt
                holder = {}

                def group(half):
                    if half == 0:
                        holder["ps"] = spp.tile(
                            [P, 2 * F], F32, tag="wd", name="qkps"
                        )
                    c0 = half * F
                    for kk in range(KD):
                        nc.tensor.matmul(
                            holder["ps"][:, c0 : c0 + F],
                            wt[:, kk, :], hT[:, kk, c0 : c0 + F],
                            start=(kk == 0), stop=(kk == KD - 1 and half == 1),
                        )

                def move():
                    nc.scalar.activation(
                        qkt[:, :], holder["ps"][:, :], AF.Identity,
                        bias=bqk_sb[:, m : m + 1],
                    )

                return group, move

            # ---- k chunk via LN bypass: W(x-mu)*rs computed as
            # (W@x - wsum (x) mu) * rs, consuming RAW x. The key-side bias
            # cancels in softmax, so this is exact -- and it frees the k/v
            # stream from the LN1 normalize chain entirely. ----
            def emit_k_bypass_group(m, mu16s):
                wt = wstream.tile([P, KD, P], BF16, tag="w", name="wt")
                nc.sync.dma_start(out=wt[:], in_=wqk[m])
                kt = qk_pool.tile([P, NTOK], BF16, tag="qkt", name="qkt")
                qk_tiles[m] = kt
                ps = spp.tile([P, 2 * F], F32, tag="wd", name="kps")
                for half in range(NHALF):
                    c0 = half * F
                    for kk in range(KD):
                        nc.tensor.matmul(
                            ps[:, c0 : c0 + F],
                            wt[:, kk, :], xt_sb[:, kk, c0 : c0 + F],
                            start=(kk == 0), stop=False,
                        )
                    nc.tensor.matmul(
                        ps[:, c0 : c0 + F],
                        wqk_rsum_sb[0:1, m * P : (m + 1) * P],
                        mu16s[half][:],
                        start=False, stop=True,
                    )
                return kt, ps

            def emit_k_bypass_move(kt_ps, rs_bs):
                kt, ps = kt_ps
                for half in range(NHALF):
                    c0 = half * F
                    nc.vector.tensor_mul(
                        kt[:, c0 : c0 + F], ps[:, c0 : c0 + F], rs_bs[half][:]
                    )

            # ---- v chunk (token-major), same bypass; the per-token rstd
            # rides the ACT Copy's scale port (v bias is folded into the
            # proj bias on the host: softmax weights sum to 1) ----
            def emit_v(t, mu16s, rs_tok):
                half = t * P // F
                trange = slice(t * P, (t + 1) * P)
                lt = slice(t * P - half * F, (t + 1) * P - half * F)
                ps = spp.tile([P, 2 * F], F32, tag="wd", name="vps")
                for c0, w in ((0, 512), (512, 256)):
                    for kk in range(KD):
                        nc.tensor.matmul(
                            ps[:, c0 : c0 + w],
                            xt_sb[:, kk, trange],
                            wv_sb[:, kk, c0 // P : (c0 + w) // P, :],
                            start=(kk == 0), stop=False,
                        )
                    nc.tensor.matmul(
                        ps[:, c0 : c0 + w],
                        mu16s[half][0:1, lt],
                        wv_rsum_sb[0:1, c0 : c0 + w],
                        start=False, stop=True,
                    )
                nc.scalar.activation(
                    v_aug[:, t, :, 0:DK],
                    ps[:, 0:D].rearrange("p (h d) -> p h d", d=DK),
                    AF.Copy,
                    scale=rs_tok[:, t : t + 1],
                )

            # DMA issue order feeds compute in need-order: xt h0 (LN1-h0),
            # pair-5 k weights, xt h1, then wv; wproj is deferred to the
            # attention phase.
            for kk in range(KD):
                nc.sync.dma_start(out=xt_sb[:, kk, 0:F], in_=xt[:, kk, 0:F])
            s0 = emit_ln_stats(xt_sb, 0, spp)
            for kk in range(KD):
                nc.sync.dma_start(
                    out=xt_sb[:, kk, F : 2 * F], in_=xt[:, kk, F : 2 * F]
                )
            if _rep == 0:
                deferred["wv_sb"] = load_const(wv)
            wv_sb = deferred["wv_sb"]
            nc.vector.memset(v_aug[:, :, :, DK : DK + 1], 1.0)

            s1 = emit_ln_stats(xt_sb, 1, spp)
            mu16s = (s0[0], s1[0])
            rs_bs = (s0[3], s1[3])

            # k and v stream off raw x while the normalize (needed only by
            # the q chunks and the interleaved pairs) completes on DVE. The
            # k-moves are emitted after the norms so the DVE queue serves
            # the q-chunk dependency (norm-h0) first.
            k5 = emit_k_bypass_group(11, mu16s)
            k0 = emit_k_bypass_group(6, mu16s)
            # transpose rstd to token-major [tok_part, chunk] for the v
            # moves. Emitted here (not right after stats) so the DRAM-bounce
            # DMAs, which wait on the rstd chain, never head-block the queue
            # entries for the k/q/prefetch weight loads.
            rs_tok = bc1.tile([P, TC], F32, tag="rt", name="rs_tok")
            for half, s in ((0, s0), (1, s1)):
                cols = slice(half * F, (half + 1) * F)
                nc.sync.dma_start(out=rs_scr[0:1, cols], in_=s[4][:])
                nc.sync.dma_start(
                    out=rs_tok[:, half * 4 : (half + 1) * 4],
                    in_=rs_scr[0:1, cols].rearrange("a (c p) -> (a p) c", p=P),
                )
            emit_v(0, mu16s, rs_tok)
            emit_v(1, mu16s, rs_tok)
            emit_ln_norm(xt_sb, hT, 0, s0[2], s0[3])
            emit_ln_norm(xt_sb, hT, 1, s1[2], s1[3])
            emit_k_bypass_move(k5, rs_bs)
            emit_k_bypass_move(k0, rs_bs)
            for t in range(2, TC):
                emit_v(t, mu16s, rs_tok)

            g_q5, mv_q5 = qk_chunk_ops(5)
            g_q5(0); g_q5(1); mv_q5()
            g_q0, mv_q0 = qk_chunk_ops(0)
            g_q0(0); g_q0(1); mv_q0()

            # Prewarm the exp table behind the last Copy move; the load
            # overlaps the first score matmuls instead of stalling the
            # mid-qkv move pipeline.
            warm1 = stat.tile([1, 1], F32, tag="st", name="warm1")
            nc.scalar.activation(warm1[:], eps_sb[:], AF.Exp)

            # Prefetch the remaining q/k chunk weights through the f1s pool
            # (idle until fc1; same tile shape, and the ring antideps line
            # up: fc1's first tiles land on slots whose attention readers
            # finish mid-attention).
            INTER = {5: (1, 7), 0: (2, 8), 1: (3, 9), 2: (4, 10)}
            wts = {}
            for w in P_ORDER[:4]:
                for m in INTER[w]:
                    wt = f1s.tile([P, KD, P], BF16, tag="f1", name="wqk_pre")
                    nc.sync.dma_start(out=wt[:], in_=wqk[m])
                    wts[m] = wt

            if _rep == 0:
                deferred["wproj_sb"] = load_const(wproj)
            wproj_sb = deferred["wproj_sb"]

            # ---- attention: double-buffered score tiles AND PV accumulators
            # so PE never couples to the ACT exp backlog ----
            spp.release()
            spA = tc.alloc_tile_pool(name="spA", bufs=2, space="PSUM")
            opsw = tc.alloc_tile_pool(name="opsw", bufs=2, space="PSUM")
            pranges = (slice(0, DK), slice(DK, P))

            # One global software pipeline over all (pair, half, kc) units:
            # S+exp for unit i, then PV for unit i-1 — no per-half drain.
            state = {}  # (j, half) -> dict(o_ps=..., pts={kc: pt})

            def emit_s_exp(j, half, kc):
                st = state.setdefault((j, half), {"pts": {}})
                if kc == 0:
                    st["o_ps"] = opsw.tile([P, 2 * F], F32, tag="ow",
                                           name="o_ps")
                q_t = qk_tiles[j]
                k_t = qk_tiles[KD + j]
                cols = slice(half * F, (half + 1) * F)
                sp = spA.tile([P, 2 * F], F32, tag="sp", name="sp")
                for hi in (0, 1):
                    pr = pranges[hi]
                    nc.tensor.matmul(
                        sp[:, hi * F : (hi + 1) * F],
                        k_t[pr, kc * P : (kc + 1) * P],
                        q_t[pr, cols],
                        start=True, stop=True,
                    )
                pt = ptp.tile([P, 2 * F], BF16, tag="pt", name="pt")
                nc.scalar.activation(
                    pt[:], sp[:], AF.Exp, scale=float(DK) ** -0.5
                )
                st["pts"][kc] = pt

            def emit_pv(j, half, kc):
                st = state[(j, half)]
                o_ps = st["o_ps"]
                pt = st["pts"].pop(kc)
                for hi in (0, 1):
                    nc.tensor.matmul(
                        o_ps[0 : DK + 1, hi * F : (hi + 1) * F],
                        v_aug[:, kc, 2 * j + hi, :],
                        pt[:, hi * F : (hi + 1) * F],
                        start=(kc == 0), stop=(kc == TC - 1),
                    )
                if kc == TC - 1:
                    # per-hi pipelined normalize: both recips (DVE), then
                    # both bcasts (Pool), then both muls (DVE) -- the in-order
                    # engine queues overlap across the two heads, halving the
                    # serial tail before the PSUM pool boundary.
                    cols = slice(half * F, (half + 1) * F)
                    recs, rbs = [], []
                    for hi in (0, 1):
                        rec = stat.tile([1, F], F32, tag="st", name="rec")
                        nc.vector.reciprocal(
                            rec[:], o_ps[DK : DK + 1, hi * F : (hi + 1) * F]
                        )
                        recs.append(rec)
                    for hi in (0, 1):
                        rec_b = bcR.tile([DK, F], F32, tag="bR", name="rec_b")
                        nc.gpsimd.partition_broadcast(rec_b[:], recs[hi][:])
                        rbs.append(rec_b)
                    for hi in (0, 1):
                        nc.vector.tensor_mul(
                            attnT[pranges[hi], j, cols],
                            o_ps[0:DK, hi * F : (hi + 1) * F],
                            rbs[hi][:],
                        )

            # Interleaved q/k chunk: 12 matmuls into a spA-ring tile fill the
            # PE while ACT works down the exp backlog; the PSUM->SBUF move
            # runs on DVE (idle during attention) so ACT never sees it.
            def emit_qk_inter(m):
                qkt = qk_pool.tile([P, NTOK], BF16, tag="qkt", name="qkt")
                qk_tiles[m] = qkt
                ps = spA.tile([P, 2 * F], F32, tag="sp", name="qkps")
                for half in range(NHALF):
                    c0 = half * F
                    for kk in range(KD):
                        nc.tensor.matmul(
                            ps[:, c0 : c0 + F],
                            wts[m][:, kk, :], hT[:, kk, c0 : c0 + F],
                            start=(kk == 0), stop=(kk == KD - 1 and half == 1),
                        )
                nc.vector.tensor_scalar_add(
                    qkt[:, :], ps[:, :], bqk_sb[:, m : m + 1]
                )

            units = [(j, half, kc)
                     for j in P_ORDER for half in range(NHALF)
                     for kc in range(TC)]
            # Emit the remaining chunks as a block here (not interleaved into
            # the unit stream: insertions starve ACT of exp backlog). The spA
            # ring + DVE moves keep them off the attention ACT queue.
            sched = {}
            for w in P_ORDER[:4]:
                for m in INTER[w]:
                    emit_qk_inter(m)

            # fc1/fc2 first-weight DMAs: emitted now (all readers of the f1s
            # slots they rotate into are recorded), so the transfers run
            # during attention and the MLP start is never DMA-paced.
            f1_tiles = []
            for m in range(KD):
                wt = f1s.tile([P, KD, P], BF16, tag="f1", name="f1w")
                nc.sync.dma_start(out=wt[:], in_=wfc1[m])
                f1_tiles.append(wt)
            w2_first = f2s.tile([P, MF1, P], BF16, tag="f2", name="f2w")
            nc.sync.dma_start(out=w2_first[:], in_=wfc2[0])
            SKEW = 1  # PE stays 1 S+exp unit ahead of the PV stream
            for i, u in enumerate(units):
                emit_s_exp(*u)
                if i >= SKEW:
                    emit_pv(*units[i - SKEW])
                if i in sched:
                    emit_qk_inter(sched[i])
            for u in units[-SKEW:]:
                emit_pv(*u)

            # Prewarm the sqrt table for LN2 behind the last exp; the load
            # overlaps proj's matmuls instead of the LN2 chain.
            warm2 = stat.tile([1, 1], F32, tag="st", name="warm2")
            nc.scalar.activation(warm2[:], eps_sb[:], AF.Sqrt)

            h_pool.release()
            vaug_pool.release()
            qk_pool.release()

            # ---- proj + residual -> x2T; contraction follows P_ORDER so the
            # last pair's normalize overlaps the first 5 chunks' matmuls.
            # m0..m3 borrow the attention pools' PSUM tiles (their per-buffer
            # antideps resolve pair-by-pair), so the pool release boundary
            # never parks the PE behind the last pair's normalize chain ----
            x2_pool = tc.alloc_tile_pool(name="x2", bufs=1, side="right")
            h2_pool = tc.alloc_tile_pool(name="h2", bufs=1, side="right")
            g_pool = tc.alloc_tile_pool(name="g", bufs=1, side="right")
            x2T = x2_pool.tile([P, KD, NTOK], BF16, name="x2T")
            h2T = h2_pool.tile([P, KD, NTOK], BF16, name="h2T")
            gT = g_pool.tile([P, MF1, NTOK], BF16, name="gT")

            def emit_proj_m(m, ps):
                for half in range(NHALF):
                    c0 = half * F
                    for i, kk in enumerate(P_ORDER):
                        nc.tensor.matmul(
                            ps[:, c0 : c0 + F],
                            wproj_sb[:, kk, m, :],
                            attnT[:, kk, c0 : c0 + F],
                            start=(i == 0), stop=(i == KD - 1),
                        )
                # LN2 squares for this chunk ride right behind the stt on
                # DVE, so LN2 stats never wait on a square backlog. The last
                # chunk's stt is split per half so its h0 lands sooner (it
                # gates the LN2-h0 sum pass).
                if m == KD - 1:
                    for half in range(NHALF):
                        c0 = half * F
                        nc.vector.scalar_tensor_tensor(
                            x2T[:, m, c0 : c0 + F], ps[:, c0 : c0 + F],
                            bproj_sb[:, m : m + 1],
                            xt_sb[:, m, c0 : c0 + F], OP.add, OP.add,
                        )
                        sq = sqp.tile([P, F], BF16, tag="sq", name="sq")
                        nc.vector.tensor_mul(
                            sq[:], x2T[:, m, c0 : c0 + F],
                            x2T[:, m, c0 : c0 + F],
                        )
                        (sq2h0 if half == 0 else sq2h1).append(sq)
                else:
                    nc.vector.scalar_tensor_tensor(
                        x2T[:, m, :], ps[:, :], bproj_sb[:, m : m + 1],
                        xt_sb[:, m, :], OP.add, OP.add,
                    )
                    for half, lst in ((0, sq2h0), (1, sq2h1)):
                        c0 = half * F
                        sq = sqp.tile([P, F], BF16, tag="sq", name="sq")
                        nc.vector.tensor_mul(
                            sq[:], x2T[:, m, c0 : c0 + F],
                            x2T[:, m, c0 : c0 + F],
                        )
                        lst.append(sq)

            sq2h0, sq2h1 = [], []
            emit_proj_m(0, spA.tile([P, 2 * F], F32, tag="sp", name="prps"))
            emit_proj_m(1, spA.tile([P, 2 * F], F32, tag="sp", name="prps"))
            emit_proj_m(2, opsw.tile([P, 2 * F], F32, tag="ow", name="prps"))
            emit_proj_m(3, opsw.tile([P, 2 * F], F32, tag="ow", name="prps"))
            emit_proj_m(4, spA.tile([P, 2 * F], F32, tag="sp", name="prps"))
            emit_proj_m(5, opsw.tile([P, 2 * F], F32, tag="ow", name="prps"))

            opsw.release()
            spA.release()
            # mlpw 3-wide + a separate 1-wide tail pool: fc2's last output
            # tile comes from the tail, so mlpw can release before the rep
            # boundary and the next rep's LN1 stats aren't gated on it.
            mlp_tail = tc.alloc_tile_pool(name="mlptl", bufs=1, space="PSUM")
            mlpw = tc.alloc_tile_pool(name="mlpw", bufs=3, space="PSUM")

            # ---- LN2 (squares were precomputed in the proj loop); h0 stats
            # on the tail pool, h1 on mlpw, so neither extends the release
            # chain that fc1's wave tiles wait on; norm-h0 sits between the
            # chains so fc1's first wave is never DVE-blocked ----
            s2h0 = emit_ln_stats(x2T, 0, mlp_tail, sq2h0)
            emit_ln_norm(x2T, h2T, 0, s2h0[2], s2h0[3])
            s2h1 = emit_ln_stats(x2T, 1, mlpw, sq2h1)
            emit_ln_norm(x2T, h2T, 1, s2h1[2], s2h1[3])

            # ---- fc1: m0..5 half-at-a-time (hides LN2-h1 latency), rest
            # full-width; two m's share one wide PSUM tile ----
            for c0 in (0, F):
                for mp in range(KD // 2):
                    ps = mlpw.tile([P, 2 * F], F32, tag="wd", name="f1ps")
                    for sub in (0, 1):
                        m = 2 * mp + sub
                        for kk in range(KD):
                            nc.tensor.matmul(
                                ps[:, sub * F : (sub + 1) * F],
                                f1_tiles[m][:, kk, :], h2T[:, kk, c0 : c0 + F],
                                start=(kk == 0), stop=(kk == KD - 1),
                            )
                    for sub in (0, 1):
                        m = 2 * mp + sub
                        nc.scalar.activation(
                            gT[:, m, c0 : c0 + F],
                            ps[:, sub * F : (sub + 1) * F],
                            AF.Gelu, bias=bfc1_sb[:, m : m + 1],
                        )
            for m in range(KD, MF1):
                wt = f1s.tile([P, KD, P], BF16, tag="f1", name="f1w")
                nc.sync.dma_start(out=wt[:], in_=wfc1[m])
                ps = mlpw.tile([P, 2 * F], F32, tag="wd", name="f1wd")
                for half in range(NHALF):
                    c0 = half * F
                    for kk in range(KD):
                        nc.tensor.matmul(
                            ps[:, c0 : c0 + F],
                            wt[:, kk, :], h2T[:, kk, c0 : c0 + F],
                            start=(kk == 0), stop=(kk == KD - 1),
                        )
                nc.scalar.activation(
                    gT[:, m, :], ps[:], AF.Gelu, bias=bfc1_sb[:, m : m + 1]
                )

            # Prewarm the sqrt table during the ACT-idle fc2 phase so the
            # next rep's LN1 Sqrt doesn't eat the table load.
            warm3 = stat.tile([1, 1], F32, tag="st", name="warm3")
            nc.scalar.activation(warm3[:], eps_sb[:], AF.Sqrt)

            # ---- fc2 + residual -> out (half-width output pieces) ----
            for m in range(KD):
                if m == 0:
                    w2 = w2_first
                else:
                    w2 = f2s.tile([P, MF1, P], BF16, tag="f2", name="f2w")
                    nc.sync.dma_start(out=w2[:], in_=wfc2[m])
                if m == KD - 1:
                    mlpw.release()
                    ps = mlp_tail.tile([P, 2 * F], F32, tag="wd", name="f2wd")
                else:
                    ps = mlpw.tile([P, 2 * F], F32, tag="wd", name="f2wd")
                for half in range(NHALF):
                    c0 = half * F
                    for kk in range(MF1):
                        nc.tensor.matmul(
                            ps[:, c0 : c0 + F],
                            w2[:, kk, :], gT[:, kk, c0 : c0 + F],
                            start=(kk == 0), stop=(kk == MF1 - 1),
                        )
                for half in range(NHALF):
                    c0 = half * F
                    yo = outp.tile([P, F], F32, tag="yo", name="yo")
                    nc.vector.scalar_tensor_tensor(
                        yo[:], ps[:, c0 : c0 + F], bfc2_sb[:, m : m + 1],
                        x2T[:, m, c0 : c0 + F], OP.add, OP.add,
                    )
                    nc.sync.dma_start(out=yt[:, m, c0 : c0 + F], in_=yo[:])

            g_pool.release()
            h2_pool.release()
            x2_pool.release()
            attn_pool.release()
            xt_pool.release()
            mlp_tail.release()

        f2s.release()
        f1s.release()
        wstream.release()
        outp.release()
        ptp.release()
        tmp.release()
        sqp.release()
        bcR.release()
        bcP.release()
        bc1.release()
        stat.release()
        const.release()

    nc.compile()
    return nc


def _retile_w(w_t, mtiles):
    """[out, in] weight -> [mtiles, P, in//P, P]: t[m,p,kk,o] = w[m*P+o, kk*P+p]."""
    out_dim, in_dim = w_t.shape
    a = w_t.reshape(mtiles, P, in_dim // P, P).transpose(0, 3, 2, 1)
    return np.ascontiguousarray(a)


def _rhs_tile(w_t):
    """[KD*P, in] weight -> [P, in//P, KD, P]: t[p,kk,m,o] = w[m*P+o, kk*P+p]."""
    a = w_t.reshape(KD, P, w_t.shape[1] // P, P).transpose(3, 2, 0, 1)
    return np.ascontiguousarray(a)


_NC_CACHE = {}


def _get_nc():
    if "nc" not in _NC_CACHE:
        _NC_CACHE["nc"] = build_program()
    return _NC_CACHE["nc"]


def prep_inputs(x, ln1_w, ln1_b, qkv_w, qkv_b, proj_w, proj_b,
                ln2_w, ln2_b, fc1_w, fc1_b, fc2_w, fc2_b):
    import ml_dtypes

    bf16 = np.dtype(ml_dtypes.bfloat16)
    f32 = lambda a: np.asarray(a, dtype=np.float32)
    x = f32(x)
    qkv_w, qkv_b = f32(qkv_w), f32(qkv_b)
    fc1_w, fc1_b = f32(fc1_w), f32(fc1_b)

    # fold LN gain/bias into the consuming layer
    wqkv_eff = qkv_w * f32(ln1_w)[None, :]
    bqkv_eff = qkv_b + qkv_w @ f32(ln1_b)
    wfc1_eff = fc1_w * f32(ln2_w)[None, :]
    bfc1_eff = fc1_b + fc1_w @ f32(ln2_b)

    # v bias is exact to fold into the proj bias: softmax weights sum to 1,
    # so attn_out = PV/denom + bv and proj(attn_out) = proj(PV/denom) +
    # proj_w @ bv + proj_b.
    bproj_eff = f32(proj_b) + f32(proj_w) @ bqkv_eff[2 * D :]

    shared = {
        "onesb": np.ones((P, 1), dtype=bf16),
        "wqk": _retile_w(wqkv_eff[: 2 * D], 2 * KD).astype(bf16),
        "wv": _rhs_tile(wqkv_eff[2 * D :]).astype(bf16),
        "bqk_pp": np.ascontiguousarray(
            bqkv_eff[: 2 * D].reshape(2 * KD, P).T
        ).astype(np.float32),
        "wqk_rsum": np.ascontiguousarray(
            -wqkv_eff[: 2 * D].sum(axis=1)[None, :]
        ).astype(bf16),
        "wv_rsum": np.ascontiguousarray(
            -wqkv_eff[2 * D :].sum(axis=1)[None, :]
        ).astype(bf16),
        "wproj": _rhs_tile(f32(proj_w)).astype(bf16),
        "bproj_pp": np.ascontiguousarray(
            bproj_eff.reshape(KD, P).T
        ).astype(np.float32),
        "wfc1": _retile_w(wfc1_eff, MF1).astype(bf16),
        "bfc1": np.ascontiguousarray(bfc1_eff.reshape(MF1, P).T),
        "wfc2": _retile_w(f32(fc2_w), KD).astype(bf16),
        "bfc2_pp": np.ascontiguousarray(
            f32(fc2_b).reshape(KD, P).T
        ).astype(np.float32),
    }
    in_maps = []
    for b in range(N_CORES):
        m = dict(shared)
        # xt[p, s, n] = x[b, n, s*P + p]
        m["xt"] = np.ascontiguousarray(
            x[b].reshape(NTOK, KD, P).transpose(2, 1, 0)
        ).astype(bf16)
        in_maps.append(m)
    return in_maps


def kernel(**inputs):
    nc = _get_nc()
    in_maps = prep_inputs(**inputs)
    res = run_bass_kernel_spmd(nc, in_maps, list(range(N_CORES)))
    outs = []
    for b in range(N_CORES):
        ytile = res.results[b]["yt"]  # [P, KD, NTOK]
        outs.append(ytile.transpose(2, 1, 0).reshape(NTOK, D))
    return np.stack(outs).astype(np.float32)
